# revision 26
# baseline (speedup 1.0000x reference)
"""GCN encoder (2-layer GCNConv + LayerNorm + ELU + residual) on 8 Trainium2
NeuronCores via Bass/Tile.

Strategy: partition nodes across the 8 cores by id (6250 each). Each core
aggregates the edges whose dst lands in its partition. The gather source is a
DRAM table h' = dinv * (x @ W) over all nodes (replicated dense compute for
layer 1; AllGather of per-core chunks for layer 2). Aggregation = dma_gather
of fp16 256B rows + PSUM-accumulating matmuls:
  - "lo" stream (table rows <= 31250): per-node slot-aligned layout, B = I
  - "hi" stream (rest, offset window for int16 idx range): densely packed,
    per-chunk staircase selection matrix built on-device with is_equal
All graph preprocessing (degrees, node ordering, slot/index layouts) happens
on host in numpy; all float math on device.
"""
import sys

sys.path.insert(0, "/opt/trn_rl_repo")

import numpy as np

N_NODES = 50000
N_EDGES = 800000
IN_DIM = 256
F = 128
NCORES = 8
NP = N_NODES // NCORES          # 6250 nodes per core
TILES = (NP + 127) // 128       # 49 dst tiles per core
NPAD = TILES * 128              # 6272
LO_ROWS = 5 * NP + 1            # 31251 rows: Z_lo row 0 + cores 0..4
HI_BASE = LO_ROWS               # hi window starts here
DTILES = (N_NODES + 127) // 128  # 391 dense tiles
DENSE_PAD = DTILES * 128        # 50048
TROWS = DENSE_PAD + 1           # table rows (row 0 = Z_lo, 1.. = positions)
CALLCOLS = 8                    # slot cols per gather call (1024 idxs)
GT = 4                          # dst tiles per batched epilogue group

_CACHE = {}


def _cumcount(key):
    """0-based running count within equal-valued runs of a sorted key."""
    n = len(key)
    if n == 0:
        return np.zeros(0, dtype=np.int64)
    first = np.r_[True, key[1:] != key[:-1]]
    start = np.maximum.accumulate(np.where(first, np.arange(n), 0))
    return np.arange(n) - start


# ----------------------------------------------------------------- host prep
def _prep(x, W1, b1, g1, be1, W2, b2, g2, be2, edge_index):
    src = np.asarray(edge_index[0], dtype=np.int64)
    dst = np.asarray(edge_index[1], dtype=np.int64)
    x = np.asarray(x)

    deg = np.bincount(dst, minlength=N_NODES).astype(np.float64) + 1.0
    dinv = (1.0 / np.sqrt(deg)).astype(np.float32)

    # self loops are NOT gathered: their contribution is added from the
    # locally-computed own rows in the epilogue
    s_all = src
    d_all = dst
    src_core = s_all // NP
    is_lo_edge = src_core <= 4

    # --- per-core ordering by lo-degree (descending) -----------------------
    # rank: position of node within its core's ordering
    rank = np.empty(N_NODES, dtype=np.int64)
    lodeg_n = np.bincount(d_all[is_lo_edge], minlength=N_NODES)
    hideg_n = np.bincount(d_all[~is_lo_edge], minlength=N_NODES)
    for c in range(NCORES):
        lo, hi = c * NP, (c + 1) * NP
        order = np.argsort(-lodeg_n[lo:hi], kind="stable")
        rank[lo + order] = np.arange(NP)
    pos = (np.arange(N_NODES) // NP) * NP + rank   # global position
    r_row = pos + 1                                # table row of each node

    # --- per-core, per-tile structure --------------------------------------
    # maxlo[c,t], hicnt[c,t]
    maxlo = np.zeros((NCORES, TILES), dtype=np.int64)
    hicnt = np.zeros((NCORES, TILES), dtype=np.int64)
    node_tile = rank // 128
    node_m = rank % 128
    for c in range(NCORES):
        lo, hi = c * NP, (c + 1) * NP
        t = node_tile[lo:hi]
        np.maximum.at(maxlo[c], t, lodeg_n[lo:hi])
        np.add.at(hicnt[c], t, hideg_n[lo:hi])
    M_lo = np.maximum(maxlo.max(axis=0), 1)               # [TILES]
    H_hi = (hicnt.max(axis=0) + 127) // 128               # [TILES] chunks

    # --- gather-call layout (shared across cores) ---------------------------
    # fixed 1024-idx calls (8 slot-columns each); SWDGE desc ring caps a
    # single dma_gather at 1024 descriptors.
    LOCUM = np.r_[0, np.cumsum(M_lo)]      # global lo col base per tile
    HICUM = np.r_[0, np.cumsum(H_hi)]      # global hi col base per tile
    LOTOT, HITOT = int(LOCUM[-1]), int(HICUM[-1])
    NCALL_LO = (LOTOT + CALLCOLS - 1) // CALLCOLS
    NCALL_HI = (HITOT + CALLCOLS - 1) // CALLCOLS
    IDXCOLS = (NCALL_LO + NCALL_HI) * CALLCOLS * 8
    HICOLS = NCALL_HI * CALLCOLS           # dstrow cols (padded)

    struct = (
        tuple(int(v) for v in M_lo),
        tuple(int(v) for v in H_hi),
    )

    # --- per-core idx + dstrow arrays --------------------------------------
    idx_arrs, dstrow_arrs, dinv_own_arrs = [], [], []
    # per-edge helper arrays
    e_dst_rank = rank[d_all]
    e_t = e_dst_rank // 128
    e_m = e_dst_rank % 128
    e_core = d_all // NP
    e_val_lo = r_row[s_all]                 # lo idx value
    e_val_hi = r_row[s_all] - HI_BASE       # hi idx value
    for c in range(NCORES):
        emask = e_core == c
        # ---------- lo stream
        lmask = emask & is_lo_edge
        lt, lm, lval = e_t[lmask], e_m[lmask], e_val_lo[lmask]
        # p-counter: order lo edges of this core by (tile, m) stably
        o = np.argsort(lt * 128 + lm, kind="stable")
        lt, lm, lval = lt[o], lm[o], lval[o]
        key = lt * 128 + lm
        p_cnt = _cumcount(key)
        # ---------- hi stream
        hmask = emask & ~is_lo_edge
        ht, hm, hval = e_t[hmask], e_m[hmask], e_val_hi[hmask]
        o = np.argsort(ht * 128 + hm, kind="stable")
        ht, hm, hval = ht[o], hm[o], hval[o]
        j_cnt = _cumcount(ht)   # within-tile dense index

        idx_big = np.zeros(IDXCOLS * 16, dtype=np.int16)
        dstrow = np.full((HICOLS * 128,), 128.0, dtype=np.float32)
        # flat slot position for stream col g, row m:
        #   1024*(g//8) + 128*(g%8) + m   (+ stream base)
        HIBASE = NCALL_LO * 1024
        g = LOCUM[lt] + p_cnt
        fl = 1024 * (g // CALLCOLS) + 128 * (g % CALLCOLS) + lm
        idx_big[fl] = lval.astype(np.int16)
        g = HICUM[ht] + j_cnt // 128
        m_slot = j_cnt % 128
        fl = HIBASE + 1024 * (g // CALLCOLS) + 128 * (g % CALLCOLS) + m_slot
        idx_big[fl] = hval.astype(np.int16)
        dstrow[g * 128 + m_slot] = hm.astype(np.float32)
        # [16, IDXCOLS] layout: slot i -> [i%16, i//16]
        idx_2d = idx_big.reshape(IDXCOLS, 16).T.copy()
        idx_arrs.append(np.tile(idx_2d, (8, 1)))
        dstrow_arrs.append(
            np.ascontiguousarray(dstrow.reshape(HICOLS, 128).T) if HICOLS else
            np.zeros((128, 1), np.float32))

        # dinv in own order [128, TILES]
        dv = np.zeros((128, TILES), np.float32)
        own = np.arange(c * NP, (c + 1) * NP)
        dv[node_m[own], node_tile[own]] = dinv[own]
        dinv_own_arrs.append(dv)

    # --- dense-phase inputs -------------------------------------------------
    order_global = np.empty(N_NODES, dtype=np.int64)
    order_global[pos] = np.arange(N_NODES)   # node id at each position
    xT = np.zeros((IN_DIM, DENSE_PAD), dtype=np.float16)
    xT[:, :N_NODES] = x[order_global].T.astype(np.float16)
    dinv_d1 = np.zeros((128, DTILES), np.float32)
    pm = np.arange(N_NODES)
    dinv_d1[pm % 128, pm // 128] = dinv[order_global]

    ln_id = (np.all(np.asarray(g1) == 1) and np.all(np.asarray(be1) == 0)
             and np.all(np.asarray(g2) == 1) and np.all(np.asarray(be2) == 0))
    rep = lambda v: np.ascontiguousarray(
        np.broadcast_to(np.asarray(v, np.float32), (128, F)))
    common = {
        "xT": xT,
        "W1": np.asarray(W1, np.float16),
        "W2h": np.asarray(W2, np.float16),
        "ident": np.eye(128, dtype=np.float16),
        "iota": np.broadcast_to(
            np.arange(128, dtype=np.float32), (128, 128)).copy(),
        "dinv_d1": dinv_d1,
        "b1r": rep(b1), "g1r": rep(g1), "be1r": rep(be1),
        "b2r": rep(b2), "g2r": rep(g2), "be2r": rep(be2),
    }
    in_maps = []
    for c in range(NCORES):
        m = dict(common)
        m["idx"] = idx_arrs[c]
        m["dstrow"] = dstrow_arrs[c]
        m["dinv_own"] = dinv_own_arrs[c]
        xo = np.zeros((IN_DIM, NPAD), dtype=np.float16)
        xo[:, :NP] = xT[:, c * NP:(c + 1) * NP]
        m["x_own"] = xo
        in_maps.append(m)
    aux = {"pos": pos, "dinv": dinv, "r_row": r_row}
    bz = bool(np.all(np.asarray(b1) == 0) and np.all(np.asarray(b2) == 0))
    struct = struct + (bool(ln_id), bz)
    return struct, in_maps, order_global, aux


# ------------------------------------------------------------- build program
# phase: 1=dense1 only, 2=+agg1 (dump y1), 3=+dense2+allgather, 4=full
# epi (debug): 0=stop after scale+bias, 1=+LN, 2=full (+ELU)
def _build(struct, phase=4, epi=2, agg_mode=2):
    import concourse.bass as bass
    import concourse.mybir as mybir
    from concourse import bacc, tile

    M_lo, H_hi, ln_id, bz = struct
    dt = mybir.dt
    AF = mybir.ActivationFunctionType
    OP = mybir.AluOpType
    LOCUM = np.r_[0, np.cumsum(M_lo)].astype(int)
    HICUM = np.r_[0, np.cumsum(H_hi)].astype(int)
    NCALL_LO = (int(LOCUM[-1]) + CALLCOLS - 1) // CALLCOLS
    NCALL_HI = (int(HICUM[-1]) + CALLCOLS - 1) // CALLCOLS
    IDXCOLS = (NCALL_LO + NCALL_HI) * CALLCOLS * 8
    HICOLS = NCALL_HI * CALLCOLS
    HIIDXBASE = NCALL_LO * CALLCOLS * 8
    HICOLS_IN = max(HICOLS, 1)

    nc = bacc.Bacc("TRN2", target_bir_lowering=False, debug=False,
                   num_devices=NCORES, num_swdge_queues=4)
    inp = lambda n, s, d: nc.dram_tensor(n, s, d, kind="ExternalInput")
    xT = inp("xT", [IN_DIM, DENSE_PAD], dt.float16)
    x_own = inp("x_own", [IN_DIM, NPAD], dt.float16)
    W1 = inp("W1", [IN_DIM, F], dt.float16)
    W2h = inp("W2h", [F, F], dt.float16)
    ident = inp("ident", [128, 128], dt.float16)
    iota = inp("iota", [128, 128], dt.float32)
    dinv_d1 = inp("dinv_d1", [128, DTILES], dt.float32)
    dinv_own = inp("dinv_own", [128, TILES], dt.float32)
    idx_in = inp("idx", [128, IDXCOLS], dt.int16)
    dstrow_in = inp("dstrow", [128, HICOLS_IN], dt.float32)
    b1r = inp("b1r", [128, F], dt.float32)
    g1r = inp("g1r", [128, F], dt.float32)
    be1r = inp("be1r", [128, F], dt.float32)
    b2r = inp("b2r", [128, F], dt.float32)
    g2r = inp("g2r", [128, F], dt.float32)
    be2r = inp("be2r", [128, F], dt.float32)
    out_own = nc.dram_tensor("out_own", [NPAD, F], dt.float32,
                             kind="ExternalOutput")

    # layer-1 table split into the two gather windows so the lo-window
    # gathers can start while dense1 is still writing the hi window
    dbg1 = "ExternalOutput" if phase == 1 else "Internal"
    HI_ROWS = TROWS - HI_BASE
    h1_lo = nc.dram_tensor("h1_lo", [LO_ROWS, F], dt.float16, kind=dbg1)
    h1_hi = nc.dram_tensor("h1_hi", [HI_ROWS, F], dt.float16, kind=dbg1)
    h2_own = nc.dram_tensor("h2_own", [NP, F], dt.float16)
    h2_dbg = (nc.dram_tensor("h2_dbg", [NP, F], dt.float16,
                             kind="ExternalOutput") if phase == 3 else None)
    h2_tab = nc.dram_tensor("h2_tab", [TROWS, F], dt.float16,
                            addr_space="Shared")

    with tile.TileContext(nc) as tc:
        cst = tc.alloc_tile_pool(name="cst", bufs=1)
        res = tc.alloc_tile_pool(name="res", bufs=1)

        ident_t = cst.tile([128, 128], dt.float16)
        nc.sync.dma_start(ident_t[:], ident[:, :])
        iota_t = cst.tile([128, 128], dt.float32)
        nc.sync.dma_start(iota_t[:], iota[:, :])
        W1_t = cst.tile([128, 2, F], dt.float16)
        for kc in range(2):
            nc.sync.dma_start(W1_t[:, kc, :], W1[kc * 128:(kc + 1) * 128, :])
        W2_t = cst.tile([128, F], dt.float16)
        nc.sync.dma_start(W2_t[:], W2h[:, :])
        dinvd1_t = cst.tile([128, DTILES], dt.float32)
        nc.sync.dma_start(dinvd1_t[:], dinv_d1[:, :])
        dinvo_t = cst.tile([128, TILES], dt.float32)
        nc.sync.dma_start(dinvo_t[:], dinv_own[:, :])
        idx_t = cst.tile([128, IDXCOLS], dt.int16)
        nc.sync.dma_start(idx_t[:], idx_in[:, :])
        dstrow_t = cst.tile([128, HICOLS_IN], dt.float32)
        nc.sync.dma_start(dstrow_t[:], dstrow_in[:, :])
        bias_ts = {}
        for nm, ap_ in (("b1", b1r), ("g1", g1r), ("be1", be1r),
                        ("b2", b2r), ("g2", g2r), ("be2", be2r)):
            t = cst.tile([128, F], dt.float32, tag=f"cst_{nm}")
            nc.sync.dma_start(t[:], ap_[:, :])
            bias_ts[nm] = t
        eps_t = cst.tile([128, 1], dt.float32)
        nc.vector.memset(eps_t[:], 1e-5)
        one_t = cst.tile([128, 1], dt.float32)
        nc.vector.memset(one_t[:], 1.0)
        zero_t = cst.tile([128, 1], dt.float32)
        nc.vector.memset(zero_t[:], 0.0)
        invF_t = cst.tile([128, 1], dt.float32)
        nc.vector.memset(invF_t[:], 1.0 / F)
        zcol_t = cst.tile([128, 128], dt.float32)
        nc.vector.memset(zcol_t[:], 0.0)
        zrow = cst.tile([128, F], dt.float16)
        nc.vector.memset(zrow[:], 0.0)
        # Z rows
        nc.sync.dma_start(h1_lo[0:1, :], zrow[:1, :])
        nc.sync.dma_start(h2_tab[0:1, :], zrow[:1, :])
        nc.sync.dma_start(h2_tab[N_NODES + 1:N_NODES + 2, :], zrow[:1, :])

        # resident accumulators for layer-1 activations
        y1_all = res.tile([128, TILES, F], dt.float32)
        y1h_all = res.tile([128, TILES, F], dt.float16)
        # locally computed own rows (self-loop contributions)
        own_h1 = res.tile([128, TILES, F], dt.float16)
        hseg2_all = res.tile([128, TILES, F], dt.float16)

        # agg pools hoisted above dense1 pools so the agg gathers carry no
        # WAR dependency on dense1's released SBUF (lo-window gathers start
        # while dense1 still writes the hi window)
        msgp = tc.alloc_tile_pool(name="msg", bufs=10)
        agp = tc.alloc_tile_pool(name="agp", bufs=3, space="PSUM")
        epp = tc.alloc_tile_pool(name="ep", bufs=3)
        d2 = d2p = None
        if phase >= 3:
            d2 = tc.alloc_tile_pool(name="d2", bufs=3)
            d2p = tc.alloc_tile_pool(name="d2p", bufs=1, space="PSUM")

        # ------------------------------------------------ dense 1: h1' table
        # XG node tiles per x-slab / per table-write DMA (HWDGE instruction
        # overhead is ~625ns, so batch DMAs hard)
        XG = 16

        def write_slab(slab, r0, nrows):
            # slab [128, XG, F] holds table rows r0 + 128*j + p
            r1 = r0 + nrows
            ap = lambda tab, a, b: tab[a:b, :].rearrange(
                "(j p) f -> p j f", p=128)
            if r1 <= LO_ROWS:
                nc.sync.dma_start(ap(h1_lo, r0, r1), slab[:, :nrows // 128, :])
            elif r0 >= LO_ROWS:
                nc.sync.dma_start(ap(h1_hi, r0 - LO_ROWS, r1 - LO_ROWS),
                                  slab[:, :nrows // 128, :])
            else:
                k = LO_ROWS - r0          # rows going to h1_lo (not 128-mult)
                kj, kr = k // 128, k % 128
                if kj:
                    nc.sync.dma_start(ap(h1_lo, r0, r0 + kj * 128),
                                      slab[:, :kj, :])
                if kr:
                    nc.sync.dma_start(h1_lo[r0 + kj * 128:LO_ROWS, :],
                                      slab[:kr, kj, :])
                    nc.sync.dma_start(h1_hi[0:128 - kr, :],
                                      slab[kr:, kj, :])
                nc.sync.dma_start(ap(h1_hi, 128 - kr, r1 - LO_ROWS),
                                  slab[:, kj + 1:nrows // 128, :])

        with (
            tc.tile_pool(name="d1", bufs=3) as d1,
            tc.tile_pool(name="d1p", bufs=3, space="PSUM") as d1p,
        ):
            # own rows first: h'_own = dinv*(x_own @ W1), kept resident for
            # the layer-1 self-loop contribution
            for g0 in range(0, TILES, 7):
                gts = range(g0, min(g0 + 7, TILES))
                ncols = 128 * len(gts)
                xo = d1.tile([128, 2, 7 * 128], dt.float16, tag="xo")
                for kc in range(2):
                    nc.sync.dma_start(
                        xo[:, kc, :ncols],
                        x_own[kc * 128:(kc + 1) * 128,
                              g0 * 128:g0 * 128 + ncols])
                for j, t in enumerate(gts):
                    ps = d1p.tile([128, F], dt.float32, tag="psd1")
                    for kc in range(2):
                        nc.tensor.matmul(
                            out=ps[:], lhsT=xo[:, kc, bass.ts(j, 128)],
                            rhs=W1_t[:, kc, :],
                            start=(kc == 0), stop=(kc == 1))
                    nc.scalar.activation(own_h1[:, t, :], ps[:], AF.Copy,
                                         scale=dinvo_t[:, t:t + 1])
            for g in range(0, DTILES, XG):
                gtiles = range(g, min(g + XG, DTILES))
                ncols = 128 * len(gtiles)
                xs = d1.tile([128, 2, ncols], dt.float16, tag="xs")
                for kc in range(2):
                    nc.sync.dma_start(
                        xs[:, kc, :],
                        xT[kc * 128:(kc + 1) * 128, g * 128:g * 128 + ncols])
                slab = d1.tile([128, len(gtiles), F], dt.float16, tag="hslab")
                for j, t in enumerate(gtiles):
                    ps = d1p.tile([128, F], dt.float32, tag="psd1")
                    for kc in range(2):
                        nc.tensor.matmul(
                            out=ps[:], lhsT=xs[:, kc, bass.ts(j, 128)],
                            rhs=W1_t[:, kc, :],
                            start=(kc == 0), stop=(kc == 1))
                    nc.scalar.activation(slab[:, j, :], ps[:], AF.Copy,
                                         scale=dinvd1_t[:, t:t + 1])
                write_slab(slab, 1 + g * 128, 128 * len(gtiles))

        # ---------------------------------------------------- aggregation fn
        def agg_layer(lo_ap, hi_ap, bname, gname, bename, post, own_ap):
            if True:
                ep = epp
                bufs = {}
                self_count = [0]

                def rhs_col(stream, g):
                    # msg slice for global stream col g; issues the 1024-idx
                    # gather call covering it on first touch. Calls rotate
                    # across the 4 SWDGE queues so their (latency-bound)
                    # 256B-row transfers overlap instead of serializing on
                    # one descriptor ring.
                    ci = g // CALLCOLS
                    key = (stream, ci)
                    if key not in bufs:
                        mt = msgp.tile([128, CALLCOLS, F], dt.float16,
                                       tag=f"m{stream}")
                        base = (0 if stream == "lo" else HIIDXBASE) \
                            + ci * CALLCOLS * 8
                        nc.gpsimd.dma_gather(
                            out_ap=mt[:],
                            in_ap=lo_ap if stream == "lo" else hi_ap,
                            idxs_ap=idx_t[:, base:base + CALLCOLS * 8],
                            num_idxs=CALLCOLS * 128,
                            num_idxs_reg=CALLCOLS * 128,
                            elem_size=F,
                            queue_num=self_count[0] % 4,
                        )
                        self_count[0] += 1
                        bufs[key] = mt
                    return bufs[key][:, g % CALLCOLS, :]

                if agg_mode == 0:      # debug: gathers only
                    for ci in range(NCALL_LO):
                        rhs_col("lo", ci * CALLCOLS)
                    for ci in range(NCALL_HI):
                        rhs_col("hi", ci * CALLCOLS)
                    return

                # wide bias/gamma/beta rows for the batched epilogue
                wide = {}
                for nm in (bname, gname, bename):
                    if (nm in (bname,) and bz) or (nm != bname and ln_id):
                        continue
                    wt = ep.tile([128, GT, F], dt.float32, tag=f"w_{nm}")
                    for g_ in range(GT):
                        nc.vector.tensor_copy(wt[:, g_, :], bias_ts[nm][:])
                    wide[nm] = wt

                for t0 in range(0, TILES, GT):
                    gts = list(range(t0, min(t0 + GT, TILES)))
                    ng = len(gts)
                    ps4 = agp.tile([128, GT, F], dt.float32, tag="psag")
                    acc4 = ep.tile([128, GT, F], dt.float32, tag="acc4")
                    for ti, t in enumerate(gts):
                        nlo, nhi = M_lo[t], H_hi[t]
                        if agg_mode == 1:
                            nhi = 0
                        # split lo columns between PE (even) and DVE (odd)
                        pe_cols = [p for p in range(nlo)
                                   if p % 2 == 0 or nlo < 3]
                        dv_cols = [p for p in range(nlo) if p not in pe_cols]
                        for i, p in enumerate(pe_cols):
                            nc.tensor.matmul(
                                out=ps4[:, ti, :], lhsT=ident_t[:],
                                rhs=rhs_col("lo", LOCUM[t] + p),
                                start=(i == 0),
                                stop=(i == len(pe_cols) - 1 and nhi == 0))
                        if dv_cols:
                            for i, p in enumerate(dv_cols):
                                if i == 0:
                                    nc.vector.tensor_copy(
                                        acc4[:, ti, :],
                                        rhs_col("lo", LOCUM[t] + p))
                                else:
                                    nc.vector.tensor_tensor(
                                        out=acc4[:, ti, :],
                                        in0=acc4[:, ti, :],
                                        in1=rhs_col("lo", LOCUM[t] + p),
                                        op=OP.add)
                        else:
                            nc.vector.tensor_copy(acc4[:, ti, :], zcol_t[:])
                        for q in range(nhi):
                            bq = ep.tile([128, 128], dt.float16, tag="bq")
                            gcol = HICUM[t] + q
                            nc.vector.tensor_tensor(
                                out=bq[:],
                                in0=dstrow_t[:, gcol:gcol + 1].to_broadcast(
                                    [128, 128]),
                                in1=iota_t[:],
                                op=OP.is_equal)
                            nc.tensor.matmul(
                                out=ps4[:, ti, :], lhsT=bq[:],
                                rhs=rhs_col("hi", gcol),
                                start=False, stop=(q == nhi - 1))

                    # ---- batched epilogue over ng tiles: self term, scale,
                    # LN, ELU — few wide instructions instead of many small
                    z = ep.tile([128, GT, F], dt.float32, tag="z")
                    nc.vector.tensor_tensor(out=z[:, 0:ng, :],
                                            in0=own_ap[:, t0:t0 + ng, :],
                                            in1=ps4[:, 0:ng, :], op=OP.add)
                    nc.vector.tensor_tensor(out=z[:, 0:ng, :],
                                            in0=z[:, 0:ng, :],
                                            in1=acc4[:, 0:ng, :], op=OP.add)
                    dinvb = dinvo_t[:, t0:t0 + ng].to_broadcast([128, ng, F])
                    nc.vector.tensor_tensor(out=z[:, 0:ng, :],
                                            in0=z[:, 0:ng, :], in1=dinvb,
                                            op=OP.mult)
                    if not bz:
                        nc.vector.tensor_tensor(out=z[:, 0:ng, :],
                                                in0=z[:, 0:ng, :],
                                                in1=wide[bname][:, 0:ng, :],
                                                op=OP.add)
                    if epi == 0:
                        post(t0, ng, z, None)
                        continue
                    s4 = ep.tile([128, GT], dt.float32, tag="s4")
                    nc.vector.reduce_sum(s4[:, 0:ng], z[:, 0:ng, :],
                                         axis=mybir.AxisListType.X)
                    zsq = ep.tile([128, GT, F], dt.float32, tag="zsq")
                    nc.vector.tensor_tensor(out=zsq[:, 0:ng, :],
                                            in0=z[:, 0:ng, :],
                                            in1=z[:, 0:ng, :], op=OP.mult)
                    ssq4 = ep.tile([128, GT], dt.float32, tag="ssq4")
                    nc.vector.reduce_sum(ssq4[:, 0:ng], zsq[:, 0:ng, :],
                                         axis=mybir.AxisListType.X)
                    mean4 = ep.tile([128, GT], dt.float32, tag="mean4")
                    nc.vector.tensor_tensor(out=mean4[:, 0:ng],
                                            in0=s4[:, 0:ng],
                                            in1=invF_t[:].to_broadcast(
                                                [128, ng]), op=OP.mult)
                    var4 = ep.tile([128, GT], dt.float32, tag="var4")
                    nc.vector.tensor_tensor(out=var4[:, 0:ng],
                                            in0=ssq4[:, 0:ng],
                                            in1=invF_t[:].to_broadcast(
                                                [128, ng]), op=OP.mult)
                    msq4 = ep.tile([128, GT], dt.float32, tag="msq4")
                    nc.vector.tensor_tensor(out=msq4[:, 0:ng],
                                            in0=mean4[:, 0:ng],
                                            in1=mean4[:, 0:ng], op=OP.mult)
                    nc.vector.tensor_tensor(out=var4[:, 0:ng],
                                            in0=var4[:, 0:ng],
                                            in1=msq4[:, 0:ng], op=OP.subtract)
                    sd4 = ep.tile([128, GT], dt.float32, tag="sd4")
                    nc.scalar.activation(sd4[:, 0:ng], var4[:, 0:ng], AF.Sqrt,
                                         bias=eps_t[:])
                    inv4 = ep.tile([128, GT], dt.float32, tag="inv4")
                    nc.vector.reciprocal(inv4[:, 0:ng], sd4[:, 0:ng])
                    zn = ep.tile([128, GT, F], dt.float32, tag="zn")
                    nc.vector.tensor_tensor(
                        out=zn[:, 0:ng, :], in0=z[:, 0:ng, :],
                        in1=mean4[:, 0:ng].to_broadcast([128, ng, F]),
                        op=OP.subtract)
                    nc.vector.tensor_tensor(
                        out=zn[:, 0:ng, :], in0=zn[:, 0:ng, :],
                        in1=inv4[:, 0:ng].to_broadcast([128, ng, F]),
                        op=OP.mult)
                    if not ln_id:
                        nc.vector.tensor_tensor(out=zn[:, 0:ng, :],
                                                in0=zn[:, 0:ng, :],
                                                in1=wide[gname][:, 0:ng, :],
                                                op=OP.mult)
                        nc.vector.tensor_tensor(out=zn[:, 0:ng, :],
                                                in0=zn[:, 0:ng, :],
                                                in1=wide[bename][:, 0:ng, :],
                                                op=OP.add)
                    if epi == 1:
                        post(t0, ng, zn, None)
                        continue
                    ex = ep.tile([128, GT, F], dt.float32, tag="ex")
                    nc.scalar.activation(ex[:, 0:ng, :], zn[:, 0:ng, :],
                                         AF.Exp)
                    oneb = one_t[:].to_broadcast([128, ng, F])
                    nc.vector.tensor_tensor(out=ex[:, 0:ng, :],
                                            in0=ex[:, 0:ng, :], in1=oneb,
                                            op=OP.min)
                    nc.vector.tensor_tensor(out=ex[:, 0:ng, :],
                                            in0=ex[:, 0:ng, :], in1=oneb,
                                            op=OP.subtract)
                    rl = ep.tile([128, GT, F], dt.float32, tag="rl")
                    nc.vector.tensor_tensor(out=rl[:, 0:ng, :],
                                            in0=zn[:, 0:ng, :],
                                            in1=zero_t[:].to_broadcast(
                                                [128, ng, F]), op=OP.max)
                    post(t0, ng, rl, ex)

        # dense-2: each tile's h2' row block is computed as soon as its y1
        # lands (collective fires right after the last tile)
        def dense2_tile(t):
            trp = d2p.tile([128, 128], dt.float16, tag="trp")
            nc.tensor.transpose(out=trp[:], in_=y1h_all[:, t, :],
                                identity=ident_t[:])
            y1T = d2.tile([128, 128], dt.float16, tag="y1T")
            nc.vector.tensor_copy(y1T[:], trp[:])
            ps2 = d2p.tile([128, F], dt.float32, tag="ps2")
            nc.tensor.matmul(out=ps2[:], lhsT=y1T[:], rhs=W2_t[:],
                             start=True, stop=True)
            nc.scalar.activation(hseg2_all[:, t, :], ps2[:], AF.Copy,
                                 scale=dinvo_t[:, t:t + 1])
            nrow = min(128, NP - t * 128)
            nc.sync.dma_start(h2_own[t * 128:t * 128 + nrow, :],
                              hseg2_all[:nrow, t, :])
            if h2_dbg is not None:
                nc.sync.dma_start(h2_dbg[t * 128:t * 128 + nrow, :],
                                  hseg2_all[:nrow, t, :])

        # layer-1 post: y1 = relu + exmin, store resident f32 + fp16
        def post1(t0, ng, rl, ex):
            if ex is None:
                nc.vector.tensor_copy(y1_all[:, t0:t0 + ng, :], rl[:, 0:ng, :])
            else:
                nc.vector.tensor_tensor(out=y1_all[:, t0:t0 + ng, :],
                                        in0=rl[:, 0:ng, :], in1=ex[:, 0:ng, :],
                                        op=mybir.AluOpType.add)
            nc.scalar.activation(y1h_all[:, t0:t0 + ng, :],
                                 y1_all[:, t0:t0 + ng, :], AF.Copy)
            if phase >= 3:
                for t in range(t0, t0 + ng):
                    dense2_tile(t)

        if phase >= 2:
            agg_layer(h1_lo[:, :], h1_hi[:, :], "b1", "g1", "be1", post1,
                      own_h1)
        if phase == 2 and agg_mode == 2:
            with tc.tile_pool(name="dbg", bufs=2) as dbg:
                for t in range(TILES):
                    yt = dbg.tile([128, F], dt.float32, tag="yt")
                    nc.vector.tensor_copy(yt[:], y1_all[:, t, :])
                    nc.sync.dma_start(out_own[t * 128:(t + 1) * 128, :], yt[:])

        # --------------------------------------------------------- AllGather
        if phase >= 3:
            nc.gpsimd.collective_compute(
                "AllGather", mybir.AluOpType.bypass,
                replica_groups=[list(range(NCORES))],
                ins=[h2_own.ap().opt()],
                outs=[h2_tab[1:N_NODES + 1, :].opt()],
            )

        # ------------------------------------------------ layer 2 + residual
        if phase >= 4:
            with tc.tile_pool(name="fin", bufs=3) as fin:
                def post2(t0, ng, rl, ex):
                    y2 = fin.tile([128, GT, F], dt.float32, tag="y2")
                    if ex is None:
                        nc.vector.tensor_copy(y2[:, 0:ng, :], rl[:, 0:ng, :])
                    else:
                        nc.vector.tensor_tensor(out=y2[:, 0:ng, :],
                                                in0=rl[:, 0:ng, :],
                                                in1=ex[:, 0:ng, :],
                                                op=mybir.AluOpType.add)
                    nc.vector.tensor_tensor(out=y2[:, 0:ng, :],
                                            in0=y2[:, 0:ng, :],
                                            in1=y1_all[:, t0:t0 + ng, :],
                                            op=mybir.AluOpType.add)
                    nc.sync.dma_start(
                        out_own[t0 * 128:(t0 + ng) * 128, :].rearrange(
                            "(g p) f -> p g f", p=128),
                        y2[:, 0:ng, :])

                agg_layer(h2_tab[0:LO_ROWS, :], h2_tab[HI_BASE:TROWS, :],
                          "b2", "g2", "be2", post2, hseg2_all)

        if d2p is not None:
            d2p.release()
            d2.release()
        epp.release()
        agp.release()
        msgp.release()
        res.release()
        cst.release()

    nc.compile()
    return nc


# ------------------------------------------------------------------- driver
def _run(inputs, trace=False, phase=4, epi=2, agg_mode=2):
    from concourse.bass_utils import run_bass_kernel_spmd

    struct, in_maps, order_global, aux = _prep(**inputs)
    key = (hash(struct), phase, epi, agg_mode)
    if key not in _CACHE:
        _CACHE[key] = _build(struct, phase=phase, epi=epi, agg_mode=agg_mode)
    nc = _CACHE[key]
    res = run_bass_kernel_spmd(nc, in_maps, core_ids=list(range(NCORES)),
                               trace=trace)
    chunks = [res.results[c]["out_own"][:NP] for c in range(NCORES)]
    out = np.empty((N_NODES, F), dtype=np.float32)
    out[order_global] = np.concatenate(chunks, axis=0)
    return out, res


def kernel(**inputs):
    out, _ = _run(inputs, trace=False)
    return out



# revision 27
# speedup vs baseline: 1.2158x; 1.2158x over previous
"""GCN encoder (2-layer GCNConv + LayerNorm + ELU + residual) on 8 Trainium2
NeuronCores via Bass/Tile.

Strategy: partition nodes across the 8 cores by id (6250 each). Each core
aggregates the edges whose dst lands in its partition. The gather source is a
DRAM table h' = dinv * (x @ W) over all nodes (replicated dense compute for
layer 1; AllGather of per-core chunks for layer 2). Aggregation = dma_gather
of fp16 256B rows + PSUM-accumulating matmuls:
  - "lo" stream (table rows <= 31250): per-node slot-aligned layout, B = I
  - "hi" stream (rest, offset window for int16 idx range): densely packed,
    per-chunk staircase selection matrix built on-device with is_equal
All graph preprocessing (degrees, node ordering, slot/index layouts) happens
on host in numpy; all float math on device.
"""
import sys

sys.path.insert(0, "/opt/trn_rl_repo")

import numpy as np

N_NODES = 50000
N_EDGES = 800000
IN_DIM = 256
F = 128
NCORES = 8
NP = N_NODES // NCORES          # 6250 nodes per core
TILES = (NP + 127) // 128       # 49 dst tiles per core
NPAD = TILES * 128              # 6272
LO_ROWS = 5 * NP + 1            # 31251 rows: Z_lo row 0 + cores 0..4
HI_BASE = LO_ROWS               # hi window starts here
DTILES = (N_NODES + 127) // 128  # 391 dense tiles
DENSE_PAD = DTILES * 128        # 50048
TROWS = DENSE_PAD + 1           # table rows (row 0 = Z_lo, 1.. = positions)
CALLCOLS = 8                    # slot cols per gather call (1024 idxs)
GT = 4                          # dst tiles per batched epilogue group

_CACHE = {}


def _cumcount(key):
    """0-based running count within equal-valued runs of a sorted key."""
    n = len(key)
    if n == 0:
        return np.zeros(0, dtype=np.int64)
    first = np.r_[True, key[1:] != key[:-1]]
    start = np.maximum.accumulate(np.where(first, np.arange(n), 0))
    return np.arange(n) - start


# ----------------------------------------------------------------- host prep
def _prep(x, W1, b1, g1, be1, W2, b2, g2, be2, edge_index):
    src = np.asarray(edge_index[0], dtype=np.int64)
    dst = np.asarray(edge_index[1], dtype=np.int64)
    x = np.asarray(x)

    deg = np.bincount(dst, minlength=N_NODES).astype(np.float64) + 1.0
    dinv = (1.0 / np.sqrt(deg)).astype(np.float32)

    # self loops are NOT gathered: their contribution is added from the
    # locally-computed own rows in the epilogue
    s_all = src
    d_all = dst
    src_core = s_all // NP
    is_lo_edge = src_core <= 4

    # --- per-core ordering by lo-degree (descending) -----------------------
    # rank: position of node within its core's ordering
    rank = np.empty(N_NODES, dtype=np.int64)
    lodeg_n = np.bincount(d_all[is_lo_edge], minlength=N_NODES)
    hideg_n = np.bincount(d_all[~is_lo_edge], minlength=N_NODES)
    for c in range(NCORES):
        lo, hi = c * NP, (c + 1) * NP
        order = np.argsort(-lodeg_n[lo:hi], kind="stable")
        rank[lo + order] = np.arange(NP)
    pos = (np.arange(N_NODES) // NP) * NP + rank   # global position
    r_row = pos + 1                                # table row of each node

    # --- per-core, per-tile structure --------------------------------------
    # maxlo[c,t], hicnt[c,t]
    maxlo = np.zeros((NCORES, TILES), dtype=np.int64)
    hicnt = np.zeros((NCORES, TILES), dtype=np.int64)
    node_tile = rank // 128
    node_m = rank % 128
    for c in range(NCORES):
        lo, hi = c * NP, (c + 1) * NP
        t = node_tile[lo:hi]
        np.maximum.at(maxlo[c], t, lodeg_n[lo:hi])
        np.add.at(hicnt[c], t, hideg_n[lo:hi])
    M_lo = np.maximum(maxlo.max(axis=0), 1)               # [TILES]
    H_hi = (hicnt.max(axis=0) + 127) // 128               # [TILES] chunks

    # --- gather-call layout (shared across cores) ---------------------------
    # fixed 1024-idx calls (8 slot-columns each); SWDGE desc ring caps a
    # single dma_gather at 1024 descriptors.
    LOCUM = np.r_[0, np.cumsum(M_lo)]      # global lo col base per tile
    HICUM = np.r_[0, np.cumsum(H_hi)]      # global hi col base per tile
    LOTOT, HITOT = int(LOCUM[-1]), int(HICUM[-1])
    NCALL_LO = (LOTOT + CALLCOLS - 1) // CALLCOLS
    NCALL_HI = (HITOT + CALLCOLS - 1) // CALLCOLS
    IDXCOLS = (NCALL_LO + NCALL_HI) * CALLCOLS * 8
    HICOLS = NCALL_HI * CALLCOLS           # dstrow cols (padded)

    struct = (
        tuple(int(v) for v in M_lo),
        tuple(int(v) for v in H_hi),
    )

    # --- per-core idx + dstrow arrays --------------------------------------
    idx_arrs, dstrow_arrs, dinv_own_arrs = [], [], []
    # per-edge helper arrays
    e_dst_rank = rank[d_all]
    e_t = e_dst_rank // 128
    e_m = e_dst_rank % 128
    e_core = d_all // NP
    e_val_lo = r_row[s_all]                 # lo idx value
    e_val_hi = r_row[s_all] - HI_BASE       # hi idx value
    for c in range(NCORES):
        emask = e_core == c
        # ---------- lo stream
        lmask = emask & is_lo_edge
        lt, lm, lval = e_t[lmask], e_m[lmask], e_val_lo[lmask]
        # p-counter: order lo edges of this core by (tile, m) stably
        o = np.argsort(lt * 128 + lm, kind="stable")
        lt, lm, lval = lt[o], lm[o], lval[o]
        key = lt * 128 + lm
        p_cnt = _cumcount(key)
        # ---------- hi stream
        hmask = emask & ~is_lo_edge
        ht, hm, hval = e_t[hmask], e_m[hmask], e_val_hi[hmask]
        o = np.argsort(ht * 128 + hm, kind="stable")
        ht, hm, hval = ht[o], hm[o], hval[o]
        j_cnt = _cumcount(ht)   # within-tile dense index

        idx_big = np.zeros(IDXCOLS * 16, dtype=np.int16)
        dstrow = np.full((HICOLS * 128,), 128.0, dtype=np.float32)
        # flat slot position for stream col g, row m:
        #   1024*(g//8) + 128*(g%8) + m   (+ stream base)
        HIBASE = NCALL_LO * 1024
        g = LOCUM[lt] + p_cnt
        fl = 1024 * (g // CALLCOLS) + 128 * (g % CALLCOLS) + lm
        idx_big[fl] = lval.astype(np.int16)
        g = HICUM[ht] + j_cnt // 128
        m_slot = j_cnt % 128
        fl = HIBASE + 1024 * (g // CALLCOLS) + 128 * (g % CALLCOLS) + m_slot
        idx_big[fl] = hval.astype(np.int16)
        dstrow[g * 128 + m_slot] = hm.astype(np.float32)
        # [16, IDXCOLS] layout: slot i -> [i%16, i//16]
        idx_2d = idx_big.reshape(IDXCOLS, 16).T.copy()
        idx_arrs.append(np.tile(idx_2d, (8, 1)))
        dstrow_arrs.append(
            np.ascontiguousarray(dstrow.reshape(HICOLS, 128).T) if HICOLS else
            np.zeros((128, 1), np.float32))

        # dinv in own order [128, TILES]
        dv = np.zeros((128, TILES), np.float32)
        own = np.arange(c * NP, (c + 1) * NP)
        dv[node_m[own], node_tile[own]] = dinv[own]
        dinv_own_arrs.append(dv)

    # --- dense-phase inputs -------------------------------------------------
    order_global = np.empty(N_NODES, dtype=np.int64)
    order_global[pos] = np.arange(N_NODES)   # node id at each position
    xT = np.zeros((IN_DIM, DENSE_PAD), dtype=np.float16)
    xT[:, :N_NODES] = x[order_global].T.astype(np.float16)
    dinv_d1 = np.zeros((128, DTILES), np.float32)
    pm = np.arange(N_NODES)
    dinv_d1[pm % 128, pm // 128] = dinv[order_global]

    ln_id = (np.all(np.asarray(g1) == 1) and np.all(np.asarray(be1) == 0)
             and np.all(np.asarray(g2) == 1) and np.all(np.asarray(be2) == 0))
    rep = lambda v: np.ascontiguousarray(
        np.broadcast_to(np.asarray(v, np.float32), (128, F)))
    common = {
        "xT": xT,
        "W1": np.asarray(W1, np.float16),
        "W2h": np.asarray(W2, np.float16),
        "ident": np.eye(128, dtype=np.float16),
        "iota": np.broadcast_to(
            np.arange(128, dtype=np.float32), (128, 128)).copy(),
        "dinv_d1": dinv_d1,
        "b1r": rep(b1), "g1r": rep(g1), "be1r": rep(be1),
        "b2r": rep(b2), "g2r": rep(g2), "be2r": rep(be2),
    }
    in_maps = []
    for c in range(NCORES):
        m = dict(common)
        m["idx"] = idx_arrs[c]
        m["dstrow"] = dstrow_arrs[c]
        m["dinv_own"] = dinv_own_arrs[c]
        xo = np.zeros((IN_DIM, NPAD), dtype=np.float16)
        xo[:, :NP] = xT[:, c * NP:(c + 1) * NP]
        m["x_own"] = xo
        in_maps.append(m)
    aux = {"pos": pos, "dinv": dinv, "r_row": r_row}
    bz = bool(np.all(np.asarray(b1) == 0) and np.all(np.asarray(b2) == 0))
    struct = struct + (bool(ln_id), bz)
    return struct, in_maps, order_global, aux


# ------------------------------------------------------------- build program
# phase: 1=dense1 only, 2=+agg1 (dump y1), 3=+dense2+allgather, 4=full
# epi (debug): 0=stop after scale+bias, 1=+LN, 2=full (+ELU)
def _build(struct, phase=4, epi=2, agg_mode=2):
    import concourse.bass as bass
    import concourse.mybir as mybir
    from concourse import bacc, tile

    M_lo, H_hi, ln_id, bz = struct
    dt = mybir.dt
    AF = mybir.ActivationFunctionType
    OP = mybir.AluOpType
    LOCUM = np.r_[0, np.cumsum(M_lo)].astype(int)
    HICUM = np.r_[0, np.cumsum(H_hi)].astype(int)
    NCALL_LO = (int(LOCUM[-1]) + CALLCOLS - 1) // CALLCOLS
    NCALL_HI = (int(HICUM[-1]) + CALLCOLS - 1) // CALLCOLS
    IDXCOLS = (NCALL_LO + NCALL_HI) * CALLCOLS * 8
    HICOLS = NCALL_HI * CALLCOLS
    HIIDXBASE = NCALL_LO * CALLCOLS * 8
    HICOLS_IN = max(HICOLS, 1)

    nc = bacc.Bacc("TRN2", target_bir_lowering=False, debug=False,
                   num_devices=NCORES, num_swdge_queues=4)
    inp = lambda n, s, d: nc.dram_tensor(n, s, d, kind="ExternalInput")
    xT = inp("xT", [IN_DIM, DENSE_PAD], dt.float16)
    x_own = inp("x_own", [IN_DIM, NPAD], dt.float16)
    W1 = inp("W1", [IN_DIM, F], dt.float16)
    W2h = inp("W2h", [F, F], dt.float16)
    ident = inp("ident", [128, 128], dt.float16)
    iota = inp("iota", [128, 128], dt.float32)
    dinv_d1 = inp("dinv_d1", [128, DTILES], dt.float32)
    dinv_own = inp("dinv_own", [128, TILES], dt.float32)
    idx_in = inp("idx", [128, IDXCOLS], dt.int16)
    dstrow_in = inp("dstrow", [128, HICOLS_IN], dt.float32)
    b1r = inp("b1r", [128, F], dt.float32)
    g1r = inp("g1r", [128, F], dt.float32)
    be1r = inp("be1r", [128, F], dt.float32)
    b2r = inp("b2r", [128, F], dt.float32)
    g2r = inp("g2r", [128, F], dt.float32)
    be2r = inp("be2r", [128, F], dt.float32)
    out_own = nc.dram_tensor("out_own", [NPAD, F], dt.float32,
                             kind="ExternalOutput")

    # layer-1 table split into the two gather windows so the lo-window
    # gathers can start while dense1 is still writing the hi window
    dbg1 = "ExternalOutput" if phase == 1 else "Internal"
    HI_ROWS = TROWS - HI_BASE
    h1_lo = nc.dram_tensor("h1_lo", [LO_ROWS, F], dt.float16, kind=dbg1)
    h1_hi = nc.dram_tensor("h1_hi", [HI_ROWS, F], dt.float16, kind=dbg1)
    h2_own = nc.dram_tensor("h2_own", [NP, F], dt.float16)
    h2_dbg = (nc.dram_tensor("h2_dbg", [NP, F], dt.float16,
                             kind="ExternalOutput") if phase == 3 else None)
    h2_tab = nc.dram_tensor("h2_tab", [TROWS, F], dt.float16,
                            addr_space="Shared")

    with tile.TileContext(nc) as tc:
        cst = tc.alloc_tile_pool(name="cst", bufs=1)
        res = tc.alloc_tile_pool(name="res", bufs=1)

        ident_t = cst.tile([128, 128], dt.float16)
        nc.sync.dma_start(ident_t[:], ident[:, :])
        iota_t = cst.tile([128, 128], dt.float32)
        nc.sync.dma_start(iota_t[:], iota[:, :])
        W1_t = cst.tile([128, 2, F], dt.float16)
        for kc in range(2):
            nc.sync.dma_start(W1_t[:, kc, :], W1[kc * 128:(kc + 1) * 128, :])
        W2_t = cst.tile([128, F], dt.float16)
        nc.sync.dma_start(W2_t[:], W2h[:, :])
        dinvd1_t = cst.tile([128, DTILES], dt.float32)
        nc.sync.dma_start(dinvd1_t[:], dinv_d1[:, :])
        dinvo_t = cst.tile([128, TILES], dt.float32)
        nc.sync.dma_start(dinvo_t[:], dinv_own[:, :])
        idx_t = cst.tile([128, IDXCOLS], dt.int16)
        nc.sync.dma_start(idx_t[:], idx_in[:, :])
        dstrow_t = cst.tile([128, HICOLS_IN], dt.float32)
        nc.sync.dma_start(dstrow_t[:], dstrow_in[:, :])
        bias_ts = {}
        for nm, ap_ in (("b1", b1r), ("g1", g1r), ("be1", be1r),
                        ("b2", b2r), ("g2", g2r), ("be2", be2r)):
            t = cst.tile([128, F], dt.float32, tag=f"cst_{nm}")
            nc.sync.dma_start(t[:], ap_[:, :])
            bias_ts[nm] = t
        eps_t = cst.tile([128, 1], dt.float32)
        nc.vector.memset(eps_t[:], 1e-5)
        one_t = cst.tile([128, 1], dt.float32)
        nc.vector.memset(one_t[:], 1.0)
        zero_t = cst.tile([128, 1], dt.float32)
        nc.vector.memset(zero_t[:], 0.0)
        invF_t = cst.tile([128, 1], dt.float32)
        nc.vector.memset(invF_t[:], 1.0 / F)
        zcol_t = cst.tile([128, 128], dt.float32)
        nc.vector.memset(zcol_t[:], 0.0)
        zrow = cst.tile([128, F], dt.float16)
        nc.vector.memset(zrow[:], 0.0)
        # Z rows
        nc.sync.dma_start(h1_lo[0:1, :], zrow[:1, :])
        nc.sync.dma_start(h2_tab[0:1, :], zrow[:1, :])
        nc.sync.dma_start(h2_tab[N_NODES + 1:N_NODES + 2, :], zrow[:1, :])

        # resident accumulators for layer-1 activations
        y1_all = res.tile([128, TILES, F], dt.float32)
        y1h_all = res.tile([128, TILES, F], dt.float16)
        # locally computed own rows (self-loop contributions)
        own_h1 = res.tile([128, TILES, F], dt.float16)
        hseg2_all = res.tile([128, TILES, F], dt.float16)

        # agg pools hoisted above dense1 pools so the agg gathers carry no
        # WAR dependency on dense1's released SBUF (lo-window gathers start
        # while dense1 still writes the hi window)
        msgp = tc.alloc_tile_pool(name="msg", bufs=10)
        agp = tc.alloc_tile_pool(name="agp", bufs=3, space="PSUM")
        epp = tc.alloc_tile_pool(name="ep", bufs=3)
        d2 = d2p = None
        if phase >= 3:
            d2 = tc.alloc_tile_pool(name="d2", bufs=3)
            d2p = tc.alloc_tile_pool(name="d2p", bufs=1, space="PSUM")

        # ------------------------------------------------ dense 1: h1' table
        # XG node tiles per x-slab / per table-write DMA (HWDGE instruction
        # overhead is ~625ns, so batch DMAs hard)
        XG = 16

        def write_slab(slab, r0, nrows):
            # slab [128, XG, F] holds table rows r0 + 128*j + p
            r1 = r0 + nrows
            ap = lambda tab, a, b: tab[a:b, :].rearrange(
                "(j p) f -> p j f", p=128)
            if r1 <= LO_ROWS:
                nc.sync.dma_start(ap(h1_lo, r0, r1), slab[:, :nrows // 128, :])
            elif r0 >= LO_ROWS:
                nc.sync.dma_start(ap(h1_hi, r0 - LO_ROWS, r1 - LO_ROWS),
                                  slab[:, :nrows // 128, :])
            else:
                k = LO_ROWS - r0          # rows going to h1_lo (not 128-mult)
                kj, kr = k // 128, k % 128
                if kj:
                    nc.sync.dma_start(ap(h1_lo, r0, r0 + kj * 128),
                                      slab[:, :kj, :])
                if kr:
                    nc.sync.dma_start(h1_lo[r0 + kj * 128:LO_ROWS, :],
                                      slab[:kr, kj, :])
                    nc.sync.dma_start(h1_hi[0:128 - kr, :],
                                      slab[kr:, kj, :])
                nc.sync.dma_start(ap(h1_hi, 128 - kr, r1 - LO_ROWS),
                                  slab[:, kj + 1:nrows // 128, :])

        with (
            tc.tile_pool(name="d1", bufs=3) as d1,
            tc.tile_pool(name="d1p", bufs=3, space="PSUM") as d1p,
        ):
            # own rows first: h'_own = dinv*(x_own @ W1), kept resident for
            # the layer-1 self-loop contribution
            for g0 in range(0, TILES, 7):
                gts = range(g0, min(g0 + 7, TILES))
                ncols = 128 * len(gts)
                xo = d1.tile([128, 2, 7 * 128], dt.float16, tag="xo")
                for kc in range(2):
                    nc.sync.dma_start(
                        xo[:, kc, :ncols],
                        x_own[kc * 128:(kc + 1) * 128,
                              g0 * 128:g0 * 128 + ncols])
                for j, t in enumerate(gts):
                    ps = d1p.tile([128, F], dt.float32, tag="psd1")
                    for kc in range(2):
                        nc.tensor.matmul(
                            out=ps[:], lhsT=xo[:, kc, bass.ts(j, 128)],
                            rhs=W1_t[:, kc, :],
                            start=(kc == 0), stop=(kc == 1))
                    nc.scalar.activation(own_h1[:, t, :], ps[:], AF.Copy,
                                         scale=dinvo_t[:, t:t + 1])
            for g in range(0, DTILES, XG):
                gtiles = range(g, min(g + XG, DTILES))
                ncols = 128 * len(gtiles)
                xs = d1.tile([128, 2, ncols], dt.float16, tag="xs")
                for kc in range(2):
                    nc.sync.dma_start(
                        xs[:, kc, :],
                        xT[kc * 128:(kc + 1) * 128, g * 128:g * 128 + ncols])
                slab = d1.tile([128, len(gtiles), F], dt.float16, tag="hslab")
                for j, t in enumerate(gtiles):
                    ps = d1p.tile([128, F], dt.float32, tag="psd1")
                    for kc in range(2):
                        nc.tensor.matmul(
                            out=ps[:], lhsT=xs[:, kc, bass.ts(j, 128)],
                            rhs=W1_t[:, kc, :],
                            start=(kc == 0), stop=(kc == 1))
                    nc.scalar.activation(slab[:, j, :], ps[:], AF.Copy,
                                         scale=dinvd1_t[:, t:t + 1])
                write_slab(slab, 1 + g * 128, 128 * len(gtiles))

        # ---------------------------------------------------- aggregation fn
        def agg_layer(lo_ap, hi_ap, bname, gname, bename, post, own_ap):
            if True:
                ep = epp
                bufs = {}
                self_count = [0]

                def rhs_col(stream, g):
                    # msg slice for global stream col g; issues the 1024-idx
                    # gather call covering it on first touch. Calls rotate
                    # across the 4 SWDGE queues so their (latency-bound)
                    # 256B-row transfers overlap instead of serializing on
                    # one descriptor ring.
                    ci = g // CALLCOLS
                    key = (stream, ci)
                    if key not in bufs:
                        mt = msgp.tile([128, CALLCOLS, F], dt.float16,
                                       tag=f"m{stream}")
                        base = (0 if stream == "lo" else HIIDXBASE) \
                            + ci * CALLCOLS * 8
                        nc.gpsimd.dma_gather(
                            out_ap=mt[:],
                            in_ap=lo_ap if stream == "lo" else hi_ap,
                            idxs_ap=idx_t[:, base:base + CALLCOLS * 8],
                            num_idxs=CALLCOLS * 128,
                            num_idxs_reg=CALLCOLS * 128,
                            elem_size=F,
                            queue_num=self_count[0] % 4,
                        )
                        self_count[0] += 1
                        bufs[key] = mt
                    return bufs[key][:, g % CALLCOLS, :]

                if agg_mode == 0:      # debug: gathers only
                    for ci in range(NCALL_LO):
                        rhs_col("lo", ci * CALLCOLS)
                    for ci in range(NCALL_HI):
                        rhs_col("hi", ci * CALLCOLS)
                    return

                # wide bias/gamma/beta rows for the batched epilogue
                wide = {}
                for nm in (bname, gname, bename):
                    if (nm in (bname,) and bz) or (nm != bname and ln_id):
                        continue
                    wt = ep.tile([128, GT, F], dt.float32, tag=f"w_{nm}")
                    for g_ in range(GT):
                        nc.vector.tensor_copy(wt[:, g_, :], bias_ts[nm][:])
                    wide[nm] = wt

                for t0 in range(0, TILES, GT):
                    gts = list(range(t0, min(t0 + GT, TILES)))
                    ng = len(gts)
                    ps4 = agp.tile([128, GT, F], dt.float32, tag="psag")
                    acc4 = ep.tile([128, GT, F], dt.float32, tag="acc4")
                    for ti, t in enumerate(gts):
                        nlo, nhi = M_lo[t], H_hi[t]
                        if agg_mode == 1:
                            nhi = 0
                        # split lo columns between PE (even) and DVE (odd)
                        pe_cols = [p for p in range(nlo)
                                   if p % 2 == 0 or nlo < 3]
                        dv_cols = [p for p in range(nlo) if p not in pe_cols]
                        for i, p in enumerate(pe_cols):
                            nc.tensor.matmul(
                                out=ps4[:, ti, :], lhsT=ident_t[:],
                                rhs=rhs_col("lo", LOCUM[t] + p),
                                start=(i == 0),
                                stop=(i == len(pe_cols) - 1 and nhi == 0))
                        if dv_cols:
                            for i, p in enumerate(dv_cols):
                                nc.vector.tensor_tensor(
                                    out=acc4[:, ti, :],
                                    in0=(zcol_t[:] if i == 0
                                         else acc4[:, ti, :]),
                                    in1=rhs_col("lo", LOCUM[t] + p),
                                    op=OP.add)
                        else:
                            nc.vector.tensor_tensor(
                                out=acc4[:, ti, :], in0=zcol_t[:],
                                in1=zcol_t[:], op=OP.add)
                        for q in range(nhi):
                            bq = ep.tile([128, 128], dt.float16, tag="bq")
                            gcol = HICUM[t] + q
                            nc.vector.tensor_tensor(
                                out=bq[:],
                                in0=dstrow_t[:, gcol:gcol + 1].to_broadcast(
                                    [128, 128]),
                                in1=iota_t[:],
                                op=OP.is_equal)
                            nc.tensor.matmul(
                                out=ps4[:, ti, :], lhsT=bq[:],
                                rhs=rhs_col("hi", gcol),
                                start=False, stop=(q == nhi - 1))

                    # ---- batched epilogue over ng tiles: self term, scale,
                    # LN, ELU — few wide instructions instead of many small
                    z = ep.tile([128, GT, F], dt.float32, tag="z")
                    nc.vector.tensor_tensor(out=z[:, 0:ng, :],
                                            in0=own_ap[:, t0:t0 + ng, :],
                                            in1=ps4[:, 0:ng, :], op=OP.add)
                    nc.vector.tensor_tensor(out=z[:, 0:ng, :],
                                            in0=z[:, 0:ng, :],
                                            in1=acc4[:, 0:ng, :], op=OP.add)
                    dinvb = dinvo_t[:, t0:t0 + ng].to_broadcast([128, ng, F])
                    nc.vector.tensor_tensor(out=z[:, 0:ng, :],
                                            in0=z[:, 0:ng, :], in1=dinvb,
                                            op=OP.mult)
                    if not bz:
                        nc.vector.tensor_tensor(out=z[:, 0:ng, :],
                                                in0=z[:, 0:ng, :],
                                                in1=wide[bname][:, 0:ng, :],
                                                op=OP.add)
                    if epi == 0:
                        post(t0, ng, z, None)
                        continue
                    s4 = ep.tile([128, GT], dt.float32, tag="s4")
                    nc.vector.reduce_sum(s4[:, 0:ng], z[:, 0:ng, :],
                                         axis=mybir.AxisListType.X)
                    zsq = ep.tile([128, GT, F], dt.float32, tag="zsq")
                    nc.vector.tensor_tensor(out=zsq[:, 0:ng, :],
                                            in0=z[:, 0:ng, :],
                                            in1=z[:, 0:ng, :], op=OP.mult)
                    ssq4 = ep.tile([128, GT], dt.float32, tag="ssq4")
                    nc.vector.reduce_sum(ssq4[:, 0:ng], zsq[:, 0:ng, :],
                                         axis=mybir.AxisListType.X)
                    mean4 = ep.tile([128, GT], dt.float32, tag="mean4")
                    nc.vector.tensor_tensor(out=mean4[:, 0:ng],
                                            in0=s4[:, 0:ng],
                                            in1=invF_t[:].to_broadcast(
                                                [128, ng]), op=OP.mult)
                    var4 = ep.tile([128, GT], dt.float32, tag="var4")
                    nc.vector.tensor_tensor(out=var4[:, 0:ng],
                                            in0=ssq4[:, 0:ng],
                                            in1=invF_t[:].to_broadcast(
                                                [128, ng]), op=OP.mult)
                    msq4 = ep.tile([128, GT], dt.float32, tag="msq4")
                    nc.vector.tensor_tensor(out=msq4[:, 0:ng],
                                            in0=mean4[:, 0:ng],
                                            in1=mean4[:, 0:ng], op=OP.mult)
                    nc.vector.tensor_tensor(out=var4[:, 0:ng],
                                            in0=var4[:, 0:ng],
                                            in1=msq4[:, 0:ng], op=OP.subtract)
                    sd4 = ep.tile([128, GT], dt.float32, tag="sd4")
                    nc.scalar.activation(sd4[:, 0:ng], var4[:, 0:ng], AF.Sqrt,
                                         bias=eps_t[:])
                    inv4 = ep.tile([128, GT], dt.float32, tag="inv4")
                    nc.vector.reciprocal(inv4[:, 0:ng], sd4[:, 0:ng])
                    zn = ep.tile([128, GT, F], dt.float32, tag="zn")
                    nc.vector.tensor_tensor(
                        out=zn[:, 0:ng, :], in0=z[:, 0:ng, :],
                        in1=mean4[:, 0:ng].to_broadcast([128, ng, F]),
                        op=OP.subtract)
                    nc.vector.tensor_tensor(
                        out=zn[:, 0:ng, :], in0=zn[:, 0:ng, :],
                        in1=inv4[:, 0:ng].to_broadcast([128, ng, F]),
                        op=OP.mult)
                    if not ln_id:
                        nc.vector.tensor_tensor(out=zn[:, 0:ng, :],
                                                in0=zn[:, 0:ng, :],
                                                in1=wide[gname][:, 0:ng, :],
                                                op=OP.mult)
                        nc.vector.tensor_tensor(out=zn[:, 0:ng, :],
                                                in0=zn[:, 0:ng, :],
                                                in1=wide[bename][:, 0:ng, :],
                                                op=OP.add)
                    if epi == 1:
                        post(t0, ng, zn, None)
                        continue
                    ex = ep.tile([128, GT, F], dt.float32, tag="ex")
                    nc.scalar.activation(ex[:, 0:ng, :], zn[:, 0:ng, :],
                                         AF.Exp)
                    oneb = one_t[:].to_broadcast([128, ng, F])
                    nc.vector.tensor_tensor(out=ex[:, 0:ng, :],
                                            in0=ex[:, 0:ng, :], in1=oneb,
                                            op=OP.min)
                    nc.vector.tensor_tensor(out=ex[:, 0:ng, :],
                                            in0=ex[:, 0:ng, :], in1=oneb,
                                            op=OP.subtract)
                    rl = ep.tile([128, GT, F], dt.float32, tag="rl")
                    nc.vector.tensor_tensor(out=rl[:, 0:ng, :],
                                            in0=zn[:, 0:ng, :],
                                            in1=zero_t[:].to_broadcast(
                                                [128, ng, F]), op=OP.max)
                    post(t0, ng, rl, ex)

        # dense-2: each tile's h2' row block is computed as soon as its y1
        # lands (collective fires right after the last tile)
        def dense2_tile(t):
            trp = d2p.tile([128, 128], dt.float16, tag="trp")
            nc.tensor.transpose(out=trp[:], in_=y1h_all[:, t, :],
                                identity=ident_t[:])
            y1T = d2.tile([128, 128], dt.float16, tag="y1T")
            nc.vector.tensor_copy(y1T[:], trp[:])
            ps2 = d2p.tile([128, F], dt.float32, tag="ps2")
            nc.tensor.matmul(out=ps2[:], lhsT=y1T[:], rhs=W2_t[:],
                             start=True, stop=True)
            nc.scalar.activation(hseg2_all[:, t, :], ps2[:], AF.Copy,
                                 scale=dinvo_t[:, t:t + 1])
            nrow = min(128, NP - t * 128)
            nc.sync.dma_start(h2_own[t * 128:t * 128 + nrow, :],
                              hseg2_all[:nrow, t, :])
            if h2_dbg is not None:
                nc.sync.dma_start(h2_dbg[t * 128:t * 128 + nrow, :],
                                  hseg2_all[:nrow, t, :])

        # layer-1 post: y1 = relu + exmin, store resident f32 + fp16
        def post1(t0, ng, rl, ex):
            if ex is None:
                nc.vector.tensor_copy(y1_all[:, t0:t0 + ng, :], rl[:, 0:ng, :])
            else:
                nc.vector.tensor_tensor(out=y1_all[:, t0:t0 + ng, :],
                                        in0=rl[:, 0:ng, :], in1=ex[:, 0:ng, :],
                                        op=mybir.AluOpType.add)
            nc.scalar.activation(y1h_all[:, t0:t0 + ng, :],
                                 y1_all[:, t0:t0 + ng, :], AF.Copy)
            if phase >= 3:
                for t in range(t0, t0 + ng):
                    dense2_tile(t)

        if phase >= 2:
            agg_layer(h1_lo[:, :], h1_hi[:, :], "b1", "g1", "be1", post1,
                      own_h1)
        if phase == 2 and agg_mode == 2:
            with tc.tile_pool(name="dbg", bufs=2) as dbg:
                for t in range(TILES):
                    yt = dbg.tile([128, F], dt.float32, tag="yt")
                    nc.vector.tensor_copy(yt[:], y1_all[:, t, :])
                    nc.sync.dma_start(out_own[t * 128:(t + 1) * 128, :], yt[:])

        # --------------------------------------------------------- AllGather
        if phase >= 3:
            nc.gpsimd.collective_compute(
                "AllGather", mybir.AluOpType.bypass,
                replica_groups=[list(range(NCORES))],
                ins=[h2_own.ap().opt()],
                outs=[h2_tab[1:N_NODES + 1, :].opt()],
            )

        # ------------------------------------------------ layer 2 + residual
        if phase >= 4:
            with tc.tile_pool(name="fin", bufs=3) as fin:
                def post2(t0, ng, rl, ex):
                    y2 = fin.tile([128, GT, F], dt.float32, tag="y2")
                    if ex is None:
                        nc.vector.tensor_copy(y2[:, 0:ng, :], rl[:, 0:ng, :])
                    else:
                        nc.vector.tensor_tensor(out=y2[:, 0:ng, :],
                                                in0=rl[:, 0:ng, :],
                                                in1=ex[:, 0:ng, :],
                                                op=mybir.AluOpType.add)
                    nc.vector.tensor_tensor(out=y2[:, 0:ng, :],
                                            in0=y2[:, 0:ng, :],
                                            in1=y1_all[:, t0:t0 + ng, :],
                                            op=mybir.AluOpType.add)
                    nc.sync.dma_start(
                        out_own[t0 * 128:(t0 + ng) * 128, :].rearrange(
                            "(g p) f -> p g f", p=128),
                        y2[:, 0:ng, :])

                agg_layer(h2_tab[0:LO_ROWS, :], h2_tab[HI_BASE:TROWS, :],
                          "b2", "g2", "be2", post2, hseg2_all)

        if d2p is not None:
            d2p.release()
            d2.release()
        epp.release()
        agp.release()
        msgp.release()
        res.release()
        cst.release()

    nc.compile()
    return nc


# ------------------------------------------------------------------- driver
def _run(inputs, trace=False, phase=4, epi=2, agg_mode=2):
    from concourse.bass_utils import run_bass_kernel_spmd

    struct, in_maps, order_global, aux = _prep(**inputs)
    key = (hash(struct), phase, epi, agg_mode)
    if key not in _CACHE:
        _CACHE[key] = _build(struct, phase=phase, epi=epi, agg_mode=agg_mode)
    nc = _CACHE[key]
    res = run_bass_kernel_spmd(nc, in_maps, core_ids=list(range(NCORES)),
                               trace=trace)
    chunks = [res.results[c]["out_own"][:NP] for c in range(NCORES)]
    out = np.empty((N_NODES, F), dtype=np.float32)
    out[order_global] = np.concatenate(chunks, axis=0)
    return out, res


def kernel(**inputs):
    out, _ = _run(inputs, trace=False)
    return out



# revision 31
# speedup vs baseline: 1.4053x; 1.1558x over previous
"""GCN encoder (2-layer GCNConv + LayerNorm + ELU + residual) on 8 Trainium2
NeuronCores via Bass/Tile.

Strategy: partition nodes across the 8 cores by id (6250 each). Each core
aggregates the edges whose dst lands in its partition. The gather source is a
DRAM table h' = dinv * (x @ W) over all nodes (replicated dense compute for
layer 1; AllGather of per-core chunks for layer 2). Aggregation = dma_gather
of fp16 256B rows + PSUM-accumulating matmuls:
  - "lo" stream (table rows <= 31250): per-node slot-aligned layout, B = I
  - "hi" stream (rest, offset window for int16 idx range): densely packed,
    per-chunk staircase selection matrix built on-device with is_equal
All graph preprocessing (degrees, node ordering, slot/index layouts) happens
on host in numpy; all float math on device.
"""
import sys

sys.path.insert(0, "/opt/trn_rl_repo")

import numpy as np

N_NODES = 50000
N_EDGES = 800000
IN_DIM = 256
F = 128
NCORES = 8
NP = N_NODES // NCORES          # 6250 nodes per core
TILES = (NP + 127) // 128       # 49 dst tiles per core
NPAD = TILES * 128              # 6272
LO_ROWS = 5 * NP + 1            # 31251 rows: Z_lo row 0 + cores 0..4
HI_BASE = LO_ROWS               # hi window starts here
DTILES = (N_NODES + 127) // 128  # 391 dense tiles
DENSE_PAD = DTILES * 128        # 50048
TROWS = DENSE_PAD + 1           # table rows (row 0 = Z_lo, 1.. = positions)
CALLCOLS = 8                    # slot cols per gather call (1024 idxs)
GT = 4                          # dst tiles per batched epilogue group

_CACHE = {}


def _cumcount(key):
    """0-based running count within equal-valued runs of a sorted key."""
    n = len(key)
    if n == 0:
        return np.zeros(0, dtype=np.int64)
    first = np.r_[True, key[1:] != key[:-1]]
    start = np.maximum.accumulate(np.where(first, np.arange(n), 0))
    return np.arange(n) - start


# ----------------------------------------------------------------- host prep
def _prep(x, W1, b1, g1, be1, W2, b2, g2, be2, edge_index):
    src = np.asarray(edge_index[0], dtype=np.int64)
    dst = np.asarray(edge_index[1], dtype=np.int64)
    x = np.asarray(x)

    deg = np.bincount(dst, minlength=N_NODES).astype(np.float64) + 1.0
    dinv = (1.0 / np.sqrt(deg)).astype(np.float32)

    # self loops are NOT gathered: their contribution is added from the
    # locally-computed own rows in the epilogue
    s_all = src
    d_all = dst
    src_core = s_all // NP
    is_lo_edge = src_core <= 4

    # --- per-core ordering by lo-degree (descending) -----------------------
    # rank: position of node within its core's ordering
    rank = np.empty(N_NODES, dtype=np.int64)
    lodeg_n = np.bincount(d_all[is_lo_edge], minlength=N_NODES)
    hideg_n = np.bincount(d_all[~is_lo_edge], minlength=N_NODES)
    for c in range(NCORES):
        lo, hi = c * NP, (c + 1) * NP
        order = np.argsort(-lodeg_n[lo:hi], kind="stable")
        rank[lo + order] = np.arange(NP)
    pos = (np.arange(N_NODES) // NP) * NP + rank   # global position
    r_row = pos + 1                                # table row of each node

    # --- per-core, per-tile structure --------------------------------------
    # maxlo[c,t], hicnt[c,t]
    maxlo = np.zeros((NCORES, TILES), dtype=np.int64)
    hicnt = np.zeros((NCORES, TILES), dtype=np.int64)
    node_tile = rank // 128
    node_m = rank % 128
    for c in range(NCORES):
        lo, hi = c * NP, (c + 1) * NP
        t = node_tile[lo:hi]
        np.maximum.at(maxlo[c], t, lodeg_n[lo:hi])
        np.add.at(hicnt[c], t, hideg_n[lo:hi])
    M_lo = np.maximum(maxlo.max(axis=0), 1)               # [TILES]
    H_hi = (hicnt.max(axis=0) + 127) // 128               # [TILES] chunks

    # --- gather-call layout (shared across cores) ---------------------------
    # fixed 1024-idx calls (8 slot-columns each); SWDGE desc ring caps a
    # single dma_gather at 1024 descriptors.
    LOCUM = np.r_[0, np.cumsum(M_lo)]      # global lo col base per tile
    HICUM = np.r_[0, np.cumsum(H_hi)]      # global hi col base per tile
    LOTOT, HITOT = int(LOCUM[-1]), int(HICUM[-1])
    NCALL_LO = (LOTOT + CALLCOLS - 1) // CALLCOLS
    NCALL_HI = (HITOT + CALLCOLS - 1) // CALLCOLS
    IDXCOLS = (NCALL_LO + NCALL_HI) * CALLCOLS * 8
    HICOLS = NCALL_HI * CALLCOLS           # dstrow cols (padded)

    struct = (
        tuple(int(v) for v in M_lo),
        tuple(int(v) for v in H_hi),
    )

    # --- per-core idx + dstrow arrays --------------------------------------
    idx_arrs, dstrow_arrs, dinv_own_arrs = [], [], []
    # per-edge helper arrays
    e_dst_rank = rank[d_all]
    e_t = e_dst_rank // 128
    e_m = e_dst_rank % 128
    e_core = d_all // NP
    e_val_lo = r_row[s_all]                 # lo idx value
    e_val_hi = r_row[s_all] - HI_BASE       # hi idx value
    for c in range(NCORES):
        emask = e_core == c
        # ---------- lo stream
        lmask = emask & is_lo_edge
        lt, lm, lval = e_t[lmask], e_m[lmask], e_val_lo[lmask]
        # p-counter: order lo edges of this core by (tile, m) stably
        o = np.argsort(lt * 128 + lm, kind="stable")
        lt, lm, lval = lt[o], lm[o], lval[o]
        key = lt * 128 + lm
        p_cnt = _cumcount(key)
        # ---------- hi stream
        hmask = emask & ~is_lo_edge
        ht, hm, hval = e_t[hmask], e_m[hmask], e_val_hi[hmask]
        o = np.argsort(ht * 128 + hm, kind="stable")
        ht, hm, hval = ht[o], hm[o], hval[o]
        j_cnt = _cumcount(ht)   # within-tile dense index

        idx_big = np.zeros(IDXCOLS * 16, dtype=np.int16)
        dstrow = np.full((HICOLS * 128,), 128.0, dtype=np.float16)
        # flat slot position for stream col g, row m:
        #   1024*(g//8) + 128*(g%8) + m   (+ stream base)
        HIBASE = NCALL_LO * 1024
        g = LOCUM[lt] + p_cnt
        fl = 1024 * (g // CALLCOLS) + 128 * (g % CALLCOLS) + lm
        idx_big[fl] = lval.astype(np.int16)
        g = HICUM[ht] + j_cnt // 128
        m_slot = j_cnt % 128
        fl = HIBASE + 1024 * (g // CALLCOLS) + 128 * (g % CALLCOLS) + m_slot
        idx_big[fl] = hval.astype(np.int16)
        dstrow[g * 128 + m_slot] = hm.astype(np.float16)
        # [16, IDXCOLS] layout: slot i -> [i%16, i//16]
        idx_2d = idx_big.reshape(IDXCOLS, 16).T.copy()
        idx_arrs.append(np.tile(idx_2d, (8, 1)))
        dstrow_arrs.append(
            np.ascontiguousarray(dstrow.reshape(HICOLS, 128).T) if HICOLS else
            np.zeros((128, 1), np.float16))

        # dinv in own order [128, TILES]
        dv = np.zeros((128, TILES), np.float32)
        own = np.arange(c * NP, (c + 1) * NP)
        dv[node_m[own], node_tile[own]] = dinv[own]
        dinv_own_arrs.append(dv)

    # --- dense-phase inputs -------------------------------------------------
    order_global = np.empty(N_NODES, dtype=np.int64)
    order_global[pos] = np.arange(N_NODES)   # node id at each position
    xT = np.zeros((IN_DIM, DENSE_PAD), dtype=np.float16)
    xT[:, :N_NODES] = x[order_global].T.astype(np.float16)
    dinv_d1 = np.zeros((128, DTILES), np.float32)
    pm = np.arange(N_NODES)
    dinv_d1[pm % 128, pm // 128] = dinv[order_global]

    ln_id = (np.all(np.asarray(g1) == 1) and np.all(np.asarray(be1) == 0)
             and np.all(np.asarray(g2) == 1) and np.all(np.asarray(be2) == 0))
    rep = lambda v: np.ascontiguousarray(
        np.broadcast_to(np.asarray(v, np.float32), (128, F)))
    common = {
        "W1": np.asarray(W1, np.float16),
        "W2h": np.asarray(W2, np.float16),
        "ident": np.eye(128, dtype=np.float16),
        "iota": np.broadcast_to(
            np.arange(128, dtype=np.float16), (128, 128)).copy(),
        "b1r": rep(b1), "g1r": rep(g1), "be1r": rep(be1),
        "b2r": rep(b2), "g2r": rep(g2), "be2r": rep(be2),
    }
    in_maps = []
    for c in range(NCORES):
        m = dict(common)
        m["idx"] = idx_arrs[c]
        m["dstrow"] = dstrow_arrs[c]
        m["dinv_own"] = dinv_own_arrs[c]
        xo = np.zeros((IN_DIM, NPAD), dtype=np.float16)
        xo[:, :NP] = xT[:, c * NP:(c + 1) * NP]
        m["x_own"] = xo
        in_maps.append(m)
    aux = {"pos": pos, "dinv": dinv, "r_row": r_row}
    bz = bool(np.all(np.asarray(b1) == 0) and np.all(np.asarray(b2) == 0))
    struct = struct + (bool(ln_id), bz)
    return struct, in_maps, order_global, aux


# ------------------------------------------------------------- build program
# phase: 1=dense1 only, 2=+agg1 (dump y1), 3=+dense2+allgather, 4=full
# epi (debug): 0=stop after scale+bias, 1=+LN, 2=full (+ELU)
def _build(struct, phase=4, epi=2, agg_mode=2):
    import concourse.bass as bass
    import concourse.mybir as mybir
    from concourse import bacc, tile

    M_lo, H_hi, ln_id, bz = struct
    dt = mybir.dt
    AF = mybir.ActivationFunctionType
    OP = mybir.AluOpType
    LOCUM = np.r_[0, np.cumsum(M_lo)].astype(int)
    HICUM = np.r_[0, np.cumsum(H_hi)].astype(int)
    NCALL_LO = (int(LOCUM[-1]) + CALLCOLS - 1) // CALLCOLS
    NCALL_HI = (int(HICUM[-1]) + CALLCOLS - 1) // CALLCOLS
    IDXCOLS = (NCALL_LO + NCALL_HI) * CALLCOLS * 8
    HICOLS = NCALL_HI * CALLCOLS
    HIIDXBASE = NCALL_LO * CALLCOLS * 8
    HICOLS_IN = max(HICOLS, 1)

    nc = bacc.Bacc("TRN2", target_bir_lowering=False, debug=False,
                   num_devices=NCORES, num_swdge_queues=4)
    inp = lambda n, s, d: nc.dram_tensor(n, s, d, kind="ExternalInput")
    x_own = inp("x_own", [IN_DIM, NPAD], dt.float16)
    W1 = inp("W1", [IN_DIM, F], dt.float16)
    W2h = inp("W2h", [F, F], dt.float16)
    ident = inp("ident", [128, 128], dt.float16)
    iota = inp("iota", [128, 128], dt.float16)
    dinv_own = inp("dinv_own", [128, TILES], dt.float32)
    idx_in = inp("idx", [128, IDXCOLS], dt.int16)
    dstrow_in = inp("dstrow", [128, HICOLS_IN], dt.float16)
    b1r = inp("b1r", [128, F], dt.float32)
    g1r = inp("g1r", [128, F], dt.float32)
    be1r = inp("be1r", [128, F], dt.float32)
    b2r = inp("b2r", [128, F], dt.float32)
    g2r = inp("g2r", [128, F], dt.float32)
    be2r = inp("be2r", [128, F], dt.float32)
    out_own = nc.dram_tensor("out_own", [NPAD, F], dt.float32,
                             kind="ExternalOutput")

    # layer-1 table: per-core shard (own_h1) AllGathered into h1_tab,
    # exactly like the layer-2 table
    h1_own = nc.dram_tensor("h1_own", [NP, F], dt.float16)
    h1_tab = nc.dram_tensor("h1_tab", [TROWS, F], dt.float16,
                            addr_space="Shared")
    h2_own = nc.dram_tensor("h2_own", [NP, F], dt.float16)
    h2_dbg = (nc.dram_tensor("h2_dbg", [NP, F], dt.float16,
                             kind="ExternalOutput") if phase == 3 else None)
    h2_tab = nc.dram_tensor("h2_tab", [TROWS, F], dt.float16,
                            addr_space="Shared")

    with tile.TileContext(nc) as tc:
        cst = tc.alloc_tile_pool(name="cst", bufs=1)
        res = tc.alloc_tile_pool(name="res", bufs=1)

        ident_t = cst.tile([128, 128], dt.float16)
        nc.sync.dma_start(ident_t[:], ident[:, :])
        iota_t = cst.tile([128, 128], dt.float16)
        nc.sync.dma_start(iota_t[:], iota[:, :])
        W1_t = cst.tile([128, 2, F], dt.float16)
        for kc in range(2):
            nc.sync.dma_start(W1_t[:, kc, :], W1[kc * 128:(kc + 1) * 128, :])
        W2_t = cst.tile([128, F], dt.float16)
        nc.sync.dma_start(W2_t[:], W2h[:, :])
        dinvo_t = cst.tile([128, TILES], dt.float32)
        nc.sync.dma_start(dinvo_t[:], dinv_own[:, :])
        idx_t = cst.tile([128, IDXCOLS], dt.int16)
        nc.sync.dma_start(idx_t[:], idx_in[:, :])
        dstrow_t = cst.tile([128, HICOLS_IN], dt.float16)
        nc.sync.dma_start(dstrow_t[:], dstrow_in[:, :])
        bias_ts = {}
        for nm, ap_ in (("b1", b1r), ("g1", g1r), ("be1", be1r),
                        ("b2", b2r), ("g2", g2r), ("be2", be2r)):
            t = cst.tile([128, F], dt.float32, tag=f"cst_{nm}")
            nc.sync.dma_start(t[:], ap_[:, :])
            bias_ts[nm] = t
        eps_t = cst.tile([128, 1], dt.float32)
        nc.vector.memset(eps_t[:], 1e-5)
        one_t = cst.tile([128, 1], dt.float32)
        nc.vector.memset(one_t[:], 1.0)
        zero_t = cst.tile([128, 1], dt.float32)
        nc.vector.memset(zero_t[:], 0.0)
        invF_t = cst.tile([128, 1], dt.float32)
        nc.vector.memset(invF_t[:], 1.0 / F)
        zcol_t = cst.tile([128, 128], dt.float32)
        nc.vector.memset(zcol_t[:], 0.0)
        zrow = cst.tile([128, F], dt.float16)
        nc.vector.memset(zrow[:], 0.0)
        # Z rows
        nc.sync.dma_start(h1_tab[0:1, :], zrow[:1, :])
        nc.sync.dma_start(h1_tab[N_NODES + 1:N_NODES + 2, :], zrow[:1, :])
        nc.sync.dma_start(h2_tab[0:1, :], zrow[:1, :])
        nc.sync.dma_start(h2_tab[N_NODES + 1:N_NODES + 2, :], zrow[:1, :])
        # B-matrix iota columns, replicated for batched is_equal
        iota_w = cst.tile([128, GT * 8, 128], dt.float16)
        for j in range(GT * 8):
            nc.vector.tensor_copy(iota_w[:, j, :], iota_t[:])

        # resident accumulators for layer-1 activations
        y1_all = res.tile([128, TILES, F], dt.float32)
        y1h_all = res.tile([128, TILES, F], dt.float16)
        # locally computed own rows (self-loop contributions)
        own_h1 = res.tile([128, TILES, F], dt.float16)
        hseg2_all = res.tile([128, TILES, F], dt.float16)

        # agg pools hoisted above dense1 pools so the agg gathers carry no
        # WAR dependency on dense1's released SBUF (lo-window gathers start
        # while dense1 still writes the hi window)
        msgp = tc.alloc_tile_pool(name="msg", bufs=10)
        agp = tc.alloc_tile_pool(name="agp", bufs=3, space="PSUM")
        epp = tc.alloc_tile_pool(name="ep", bufs=3)
        d2 = d2p = None
        if phase >= 3:
            d2 = tc.alloc_tile_pool(name="d2", bufs=3)
            d2p = tc.alloc_tile_pool(name="d2p", bufs=1, space="PSUM")

        # ------------------------- dense 1 (sharded): own rows -> AllGather
        # h'_own = dinv*(x_own @ W1) is both this core's table shard and the
        # layer-1 self-loop contribution
        with (
            tc.tile_pool(name="d1", bufs=3) as d1,
            tc.tile_pool(name="d1p", bufs=3, space="PSUM") as d1p,
        ):
            for g0 in range(0, TILES, 7):
                gts = range(g0, min(g0 + 7, TILES))
                ncols = 128 * len(gts)
                xo = d1.tile([128, 2, 7 * 128], dt.float16, tag="xo")
                for kc in range(2):
                    nc.sync.dma_start(
                        xo[:, kc, :ncols],
                        x_own[kc * 128:(kc + 1) * 128,
                              g0 * 128:g0 * 128 + ncols])
                for j, t in enumerate(gts):
                    ps = d1p.tile([128, F], dt.float32, tag="psd1")
                    for kc in range(2):
                        nc.tensor.matmul(
                            out=ps[:], lhsT=xo[:, kc, bass.ts(j, 128)],
                            rhs=W1_t[:, kc, :],
                            start=(kc == 0), stop=(kc == 1))
                    nc.scalar.activation(own_h1[:, t, :], ps[:], AF.Copy,
                                         scale=dinvo_t[:, t:t + 1])
                nrow = min(NP, (g0 + len(gts)) * 128) - g0 * 128
                jfull, rpart = nrow // 128, nrow % 128
                if jfull:
                    nc.sync.dma_start(
                        h1_own[g0 * 128:g0 * 128 + jfull * 128, :].rearrange(
                            "(j p) f -> p j f", p=128),
                        own_h1[:, g0:g0 + jfull, :])
                if rpart:
                    nc.sync.dma_start(
                        h1_own[g0 * 128 + jfull * 128:g0 * 128 + nrow, :],
                        own_h1[:rpart, g0 + jfull, :])

        nc.gpsimd.collective_compute(
            "AllGather", mybir.AluOpType.bypass,
            replica_groups=[list(range(NCORES))],
            ins=[h1_own.ap().opt()],
            outs=[h1_tab[1:N_NODES + 1, :].opt()],
        )

        # ---------------------------------------------------- aggregation fn
        def agg_layer(lo_ap, hi_ap, bname, gname, bename, post, own_ap):
            if True:
                ep = epp
                bufs = {}
                self_count = [0]

                def rhs_col(stream, g):
                    # msg slice for global stream col g; issues the 1024-idx
                    # gather call covering it on first touch. Calls rotate
                    # across the 4 SWDGE queues so their (latency-bound)
                    # 256B-row transfers overlap instead of serializing on
                    # one descriptor ring.
                    ci = g // CALLCOLS
                    key = (stream, ci)
                    if key not in bufs:
                        mt = msgp.tile([128, CALLCOLS, F], dt.float16,
                                       tag=f"m{stream}")
                        base = (0 if stream == "lo" else HIIDXBASE) \
                            + ci * CALLCOLS * 8
                        nc.gpsimd.dma_gather(
                            out_ap=mt[:],
                            in_ap=lo_ap if stream == "lo" else hi_ap,
                            idxs_ap=idx_t[:, base:base + CALLCOLS * 8],
                            num_idxs=CALLCOLS * 128,
                            num_idxs_reg=CALLCOLS * 128,
                            elem_size=F,
                            queue_num=self_count[0] % 4,
                        )
                        self_count[0] += 1
                        bufs[key] = mt
                    return bufs[key][:, g % CALLCOLS, :]

                if agg_mode == 0:      # debug: gathers only
                    for ci in range(NCALL_LO):
                        rhs_col("lo", ci * CALLCOLS)
                    for ci in range(NCALL_HI):
                        rhs_col("hi", ci * CALLCOLS)
                    return

                # wide bias/gamma/beta rows for the batched epilogue
                wide = {}
                for nm in (bname, gname, bename):
                    if (nm in (bname,) and bz) or (nm != bname and ln_id):
                        continue
                    wt = ep.tile([128, GT, F], dt.float32, tag=f"w_{nm}")
                    for g_ in range(GT):
                        nc.vector.tensor_copy(wt[:, g_, :], bias_ts[nm][:])
                    wide[nm] = wt

                for t0 in range(0, TILES, GT):
                    gts = list(range(t0, min(t0 + GT, TILES)))
                    ng = len(gts)
                    ps4 = agp.tile([128, GT, F], dt.float32, tag="psag")
                    acc4 = ep.tile([128, GT, F], dt.float32, tag="acc4")
                    h0, h1c = int(HICUM[gts[0]]), int(HICUM[gts[-1] + 1])
                    nh_g = h1c - h0
                    bqa = None
                    if nh_g and agg_mode != 1:
                        bqa = ep.tile([128, GT * 8, 128], dt.float16,
                                      tag="bqa")
                        nc.vector.tensor_tensor(
                            out=bqa[:, 0:nh_g, :],
                            in0=dstrow_t[:, h0:h0 + nh_g].to_broadcast(
                                [128, nh_g, 128]),
                            in1=iota_w[:, 0:nh_g, :],
                            op=OP.is_equal)
                    for ti, t in enumerate(gts):
                        nlo, nhi = M_lo[t], H_hi[t]
                        if agg_mode == 1:
                            nhi = 0
                        # split lo columns between PE (even) and DVE (odd)
                        pe_cols = [p for p in range(nlo)
                                   if p % 2 == 0 or nlo < 3]
                        dv_cols = [p for p in range(nlo) if p not in pe_cols]
                        for i, p in enumerate(pe_cols):
                            nc.tensor.matmul(
                                out=ps4[:, ti, :], lhsT=ident_t[:],
                                rhs=rhs_col("lo", LOCUM[t] + p),
                                start=(i == 0),
                                stop=(i == len(pe_cols) - 1 and nhi == 0))
                        if dv_cols:
                            for i, p in enumerate(dv_cols):
                                nc.vector.tensor_tensor(
                                    out=acc4[:, ti, :],
                                    in0=(zcol_t[:] if i == 0
                                         else acc4[:, ti, :]),
                                    in1=rhs_col("lo", LOCUM[t] + p),
                                    op=OP.add)
                        else:
                            nc.vector.tensor_tensor(
                                out=acc4[:, ti, :], in0=zcol_t[:],
                                in1=zcol_t[:], op=OP.add)
                        for q in range(nhi):
                            gcol = HICUM[t] + q
                            nc.tensor.matmul(
                                out=ps4[:, ti, :],
                                lhsT=bqa[:, gcol - h0, :],
                                rhs=rhs_col("hi", gcol),
                                start=False, stop=(q == nhi - 1))

                    # ---- batched epilogue over ng tiles: self term, scale,
                    # LN, ELU — few wide instructions instead of many small
                    z = ep.tile([128, GT, F], dt.float32, tag="z")
                    nc.vector.tensor_tensor(out=z[:, 0:ng, :],
                                            in0=own_ap[:, t0:t0 + ng, :],
                                            in1=ps4[:, 0:ng, :], op=OP.add)
                    nc.vector.tensor_tensor(out=z[:, 0:ng, :],
                                            in0=z[:, 0:ng, :],
                                            in1=acc4[:, 0:ng, :], op=OP.add)
                    dinvb = dinvo_t[:, t0:t0 + ng].to_broadcast([128, ng, F])
                    nc.vector.tensor_tensor(out=z[:, 0:ng, :],
                                            in0=z[:, 0:ng, :], in1=dinvb,
                                            op=OP.mult)
                    if not bz:
                        nc.vector.tensor_tensor(out=z[:, 0:ng, :],
                                                in0=z[:, 0:ng, :],
                                                in1=wide[bname][:, 0:ng, :],
                                                op=OP.add)
                    if epi == 0:
                        post(t0, ng, z, None)
                        continue
                    s4 = ep.tile([128, GT], dt.float32, tag="s4")
                    nc.vector.reduce_sum(s4[:, 0:ng], z[:, 0:ng, :],
                                         axis=mybir.AxisListType.X)
                    zsq = ep.tile([128, GT, F], dt.float32, tag="zsq")
                    nc.vector.tensor_tensor(out=zsq[:, 0:ng, :],
                                            in0=z[:, 0:ng, :],
                                            in1=z[:, 0:ng, :], op=OP.mult)
                    ssq4 = ep.tile([128, GT], dt.float32, tag="ssq4")
                    nc.vector.reduce_sum(ssq4[:, 0:ng], zsq[:, 0:ng, :],
                                         axis=mybir.AxisListType.X)
                    mean4 = ep.tile([128, GT], dt.float32, tag="mean4")
                    nc.vector.tensor_tensor(out=mean4[:, 0:ng],
                                            in0=s4[:, 0:ng],
                                            in1=invF_t[:].to_broadcast(
                                                [128, ng]), op=OP.mult)
                    var4 = ep.tile([128, GT], dt.float32, tag="var4")
                    nc.vector.tensor_tensor(out=var4[:, 0:ng],
                                            in0=ssq4[:, 0:ng],
                                            in1=invF_t[:].to_broadcast(
                                                [128, ng]), op=OP.mult)
                    msq4 = ep.tile([128, GT], dt.float32, tag="msq4")
                    nc.vector.tensor_tensor(out=msq4[:, 0:ng],
                                            in0=mean4[:, 0:ng],
                                            in1=mean4[:, 0:ng], op=OP.mult)
                    nc.vector.tensor_tensor(out=var4[:, 0:ng],
                                            in0=var4[:, 0:ng],
                                            in1=msq4[:, 0:ng], op=OP.subtract)
                    sd4 = ep.tile([128, GT], dt.float32, tag="sd4")
                    nc.scalar.activation(sd4[:, 0:ng], var4[:, 0:ng], AF.Sqrt,
                                         bias=eps_t[:])
                    inv4 = ep.tile([128, GT], dt.float32, tag="inv4")
                    nc.vector.reciprocal(inv4[:, 0:ng], sd4[:, 0:ng])
                    zn = ep.tile([128, GT, F], dt.float32, tag="zn")
                    nc.vector.tensor_tensor(
                        out=zn[:, 0:ng, :], in0=z[:, 0:ng, :],
                        in1=mean4[:, 0:ng].to_broadcast([128, ng, F]),
                        op=OP.subtract)
                    nc.vector.tensor_tensor(
                        out=zn[:, 0:ng, :], in0=zn[:, 0:ng, :],
                        in1=inv4[:, 0:ng].to_broadcast([128, ng, F]),
                        op=OP.mult)
                    if not ln_id:
                        nc.vector.tensor_tensor(out=zn[:, 0:ng, :],
                                                in0=zn[:, 0:ng, :],
                                                in1=wide[gname][:, 0:ng, :],
                                                op=OP.mult)
                        nc.vector.tensor_tensor(out=zn[:, 0:ng, :],
                                                in0=zn[:, 0:ng, :],
                                                in1=wide[bename][:, 0:ng, :],
                                                op=OP.add)
                    if epi == 1:
                        post(t0, ng, zn, None)
                        continue
                    ex = ep.tile([128, GT, F], dt.float32, tag="ex")
                    nc.scalar.activation(ex[:, 0:ng, :], zn[:, 0:ng, :],
                                         AF.Exp)
                    oneb = one_t[:].to_broadcast([128, ng, F])
                    nc.vector.tensor_tensor(out=ex[:, 0:ng, :],
                                            in0=ex[:, 0:ng, :], in1=oneb,
                                            op=OP.min)
                    nc.vector.tensor_tensor(out=ex[:, 0:ng, :],
                                            in0=ex[:, 0:ng, :], in1=oneb,
                                            op=OP.subtract)
                    rl = ep.tile([128, GT, F], dt.float32, tag="rl")
                    nc.vector.tensor_tensor(out=rl[:, 0:ng, :],
                                            in0=zn[:, 0:ng, :],
                                            in1=zero_t[:].to_broadcast(
                                                [128, ng, F]), op=OP.max)
                    post(t0, ng, rl, ex)

        # dense-2: each tile's h2' row block is computed as soon as its y1
        # lands (collective fires right after the last tile)
        def dense2_tile(t):
            trp = d2p.tile([128, 128], dt.float16, tag="trp")
            nc.tensor.transpose(out=trp[:], in_=y1h_all[:, t, :],
                                identity=ident_t[:])
            y1T = d2.tile([128, 128], dt.float16, tag="y1T")
            nc.vector.tensor_copy(y1T[:], trp[:])
            ps2 = d2p.tile([128, F], dt.float32, tag="ps2")
            nc.tensor.matmul(out=ps2[:], lhsT=y1T[:], rhs=W2_t[:],
                             start=True, stop=True)
            nc.scalar.activation(hseg2_all[:, t, :], ps2[:], AF.Copy,
                                 scale=dinvo_t[:, t:t + 1])
            nrow = min(128, NP - t * 128)
            nc.sync.dma_start(h2_own[t * 128:t * 128 + nrow, :],
                              hseg2_all[:nrow, t, :])
            if h2_dbg is not None:
                nc.sync.dma_start(h2_dbg[t * 128:t * 128 + nrow, :],
                                  hseg2_all[:nrow, t, :])

        # layer-1 post: y1 = relu + exmin, store resident f32 + fp16
        def post1(t0, ng, rl, ex):
            if ex is None:
                nc.vector.tensor_copy(y1_all[:, t0:t0 + ng, :], rl[:, 0:ng, :])
            else:
                nc.vector.tensor_tensor(out=y1_all[:, t0:t0 + ng, :],
                                        in0=rl[:, 0:ng, :], in1=ex[:, 0:ng, :],
                                        op=mybir.AluOpType.add)
            nc.scalar.activation(y1h_all[:, t0:t0 + ng, :],
                                 y1_all[:, t0:t0 + ng, :], AF.Copy)
            if phase >= 3:
                for t in range(t0, t0 + ng):
                    dense2_tile(t)

        if phase >= 2:
            agg_layer(h1_tab[0:LO_ROWS, :], h1_tab[HI_BASE:TROWS, :],
                      "b1", "g1", "be1", post1,
                      own_h1)
        if phase == 2 and agg_mode == 2:
            with tc.tile_pool(name="dbg", bufs=2) as dbg:
                for t in range(TILES):
                    yt = dbg.tile([128, F], dt.float32, tag="yt")
                    nc.vector.tensor_copy(yt[:], y1_all[:, t, :])
                    nc.sync.dma_start(out_own[t * 128:(t + 1) * 128, :], yt[:])

        # --------------------------------------------------------- AllGather
        if phase >= 3:
            nc.gpsimd.collective_compute(
                "AllGather", mybir.AluOpType.bypass,
                replica_groups=[list(range(NCORES))],
                ins=[h2_own.ap().opt()],
                outs=[h2_tab[1:N_NODES + 1, :].opt()],
            )

        # ------------------------------------------------ layer 2 + residual
        if phase >= 4:
            with tc.tile_pool(name="fin", bufs=3) as fin:
                def post2(t0, ng, rl, ex):
                    y2 = fin.tile([128, GT, F], dt.float32, tag="y2")
                    if ex is None:
                        nc.vector.tensor_copy(y2[:, 0:ng, :], rl[:, 0:ng, :])
                    else:
                        nc.vector.tensor_tensor(out=y2[:, 0:ng, :],
                                                in0=rl[:, 0:ng, :],
                                                in1=ex[:, 0:ng, :],
                                                op=mybir.AluOpType.add)
                    nc.vector.tensor_tensor(out=y2[:, 0:ng, :],
                                            in0=y2[:, 0:ng, :],
                                            in1=y1_all[:, t0:t0 + ng, :],
                                            op=mybir.AluOpType.add)
                    nc.sync.dma_start(
                        out_own[t0 * 128:(t0 + ng) * 128, :].rearrange(
                            "(g p) f -> p g f", p=128),
                        y2[:, 0:ng, :])

                agg_layer(h2_tab[0:LO_ROWS, :], h2_tab[HI_BASE:TROWS, :],
                          "b2", "g2", "be2", post2, hseg2_all)

        if d2p is not None:
            d2p.release()
            d2.release()
        epp.release()
        agp.release()
        msgp.release()
        res.release()
        cst.release()

    nc.compile()
    return nc


# ------------------------------------------------------------------- driver
def _run(inputs, trace=False, phase=4, epi=2, agg_mode=2):
    from concourse.bass_utils import run_bass_kernel_spmd

    struct, in_maps, order_global, aux = _prep(**inputs)
    key = (hash(struct), phase, epi, agg_mode)
    if key not in _CACHE:
        _CACHE[key] = _build(struct, phase=phase, epi=epi, agg_mode=agg_mode)
    nc = _CACHE[key]
    res = run_bass_kernel_spmd(nc, in_maps, core_ids=list(range(NCORES)),
                               trace=trace)
    chunks = [res.results[c]["out_own"][:NP] for c in range(NCORES)]
    out = np.empty((N_NODES, F), dtype=np.float32)
    out[order_global] = np.concatenate(chunks, axis=0)
    return out, res


def kernel(**inputs):
    out, _ = _run(inputs, trace=False)
    return out

